# revision 1
# baseline (speedup 1.0000x reference)
"""BitNetV3 transformer block on 8 Trainium2 NeuronCores.

Sharding: sequence-parallel. Each core owns 512 query tokens (two
256-token blocks (g, g+4) of one batch element; cores 0-3 -> batch 0,
cores 4-7 -> batch 1). Weights are replicated, host-pre-transposed and
bf16-cast so every matmul's stationary operand DMAs naturally. K/V are
computed per-core for owned tokens and exchanged with two 4-rank
AllGathers (replica groups {0-3}, {4-7}). Causal masking uses
host-supplied per-core 0/1 mask tiles so the SPMD program is identical
on every core. Activations live transposed ([d, token]) end to end;
per-token reductions (rmsnorm stats, softmax denominators) use
ones-vector matmuls onto partition 0 + gpsimd partition_broadcast.

Each core returns its tokens' [128, 16, 512] slice; the host
reassembles the full (2, 2048, 2048) output.
"""

import os
from contextlib import ExitStack

import numpy as np
import ml_dtypes

# ---- problem constants (hardcoded per the harness contract) ----
B = 2
S = 2048
D = 2048
H = 16
HD = 128
DFF = 8192
EPS = 1e-6
ISQ = float(1.0 / np.sqrt(HD))

P = 128  # partitions
KO = D // P  # 16 d-tiles
Q = 512  # tokens per core
NB = S // P  # 16 k-tiles per batch
MF = DFF // P  # 64 dff-tiles
BLK = 256  # token block
NCORES = 8

BF16 = ml_dtypes.bfloat16


# ---------------------------------------------------------------------------
# Host-side data preparation (sharding + layout)
# ---------------------------------------------------------------------------

def _w5(w_t: np.ndarray, kt: int, mt: int) -> np.ndarray:
    """[K, M] (transposed weight, contraction-major) -> [128, mt, kt, 128]
    with W5[p, m, k, i] = w_t[k*128+p, m*128+i]."""
    K, M = w_t.shape
    assert K == kt * P and M == mt * P
    return np.ascontiguousarray(
        w_t.reshape(kt, P, mt, P).transpose(1, 2, 0, 3)
    ).astype(BF16)


def _core_tokens(g: int) -> np.ndarray:
    """Global (within-batch) token indices for group-rank g: blocks g, g+4."""
    t1 = np.arange(BLK * g, BLK * (g + 1))
    t2 = np.arange(BLK * (g + 4), BLK * (g + 5))
    return np.concatenate([t1, t2])


def _core_mask(g: int) -> np.ndarray:
    """[128, NB, 512] bf16 causal mask in the device layout.

    jj < 8: cols 0..511 = all 512 owned q tokens.
    jj >= 8: cols 0..255 = q tokens of block g+4 (shifted layout)."""
    toks = _core_tokens(g)  # 512 global q indices
    m = np.zeros((P, NB, Q), dtype=np.float32)
    for jj in range(NB):
        kk = 128 * jj + np.arange(P)  # global k indices of this tile
        if jj < 8:
            m[:, jj, :] = kk[:, None] <= toks[None, :]
        else:
            m[:, jj, :BLK] = kk[:, None] <= toks[None, BLK:]
    return m.astype(BF16)


def prepare_shared(wq, wk, wv, wo, w_gate, w_up, w_down, ln1_w, ln2_w):
    d = {}
    d["wqt"] = _w5(np.ascontiguousarray(wq.T), KO, KO)
    d["wkt"] = _w5(np.ascontiguousarray(wk.T), KO, KO)
    d["wot"] = _w5(np.ascontiguousarray(wo.T), KO, KO)
    # V projection rhs layout: [128, ko, 2048]
    d["wvt"] = np.ascontiguousarray(
        wv.T.reshape(KO, P, D).transpose(1, 0, 2)
    ).astype(BF16)
    d["wgt"] = _w5(np.ascontiguousarray(w_gate.T), KO, MF)
    d["wut"] = _w5(np.ascontiguousarray(w_up.T), KO, MF)
    d["wdt"] = _w5(np.ascontiguousarray(w_down.T), MF, KO)
    d["ln1"] = np.ascontiguousarray(ln1_w.reshape(KO, P).T).astype(np.float32)
    d["ln2"] = np.ascontiguousarray(ln2_w.reshape(KO, P).T).astype(np.float32)
    return d


def prepare_core(hidden, core: int):
    b, g = core // 4, core % 4
    toks = _core_tokens(g)
    ht = hidden[b][toks].T  # [2048 d, 512 q]
    ht5 = np.ascontiguousarray(ht.reshape(KO, P, Q).transpose(1, 0, 2)).astype(
        np.float32
    )
    return {"ht": ht5, "mask": _core_mask(g)}


def assemble(outs, hidden_dtype):
    """outs: list of per-core [128, KO, 512] fp32 -> full (B, S, D)."""
    full = np.empty((B, S, D), dtype=np.float32)
    for core in range(NCORES):
        b, g = core // 4, core % 4
        toks = _core_tokens(g)
        o = np.asarray(outs[core])  # [p, ko, q]
        full[b, toks, :] = o.transpose(2, 1, 0).reshape(Q, D)
    return full.astype(hidden_dtype)


def _agcol(jj: int) -> int:
    """Global k-tile jj -> row/col offset in the rank-major AG buffers."""
    j = jj // 2  # 256-token block index
    return 512 * (j % 4) + 256 * (j // 4) + 128 * (jj % 2)


# ---------------------------------------------------------------------------
# Pure-numpy simulation of the exact device dataflow (for fast validation)
# ---------------------------------------------------------------------------

def _bf(x):
    return x.astype(BF16).astype(np.float32)


def _sim_norm(ht, lnw):
    # ht: [2048, 512] fp32 (d, q); lnw: [2048]
    sq = _bf(_bf(ht) * _bf(ht))  # DVE squares, bf16 out
    ms = sq.sum(axis=0)  # PE ones-matmul, fp32 accum
    rstd = 1.0 / np.sqrt(ms / D + EPS)
    return _bf((ht * rstd[None, :]) * lnw[:, None])  # bf16 out


def host_simulate(inputs):
    """Numpy replica of the device algorithm, including AG layout and masks."""
    hidden = np.asarray(inputs["hidden_states"], dtype=np.float32)
    f32 = lambda k: np.asarray(inputs[k], dtype=np.float32)  # noqa: E731
    wqT, wkT, wvT, woT = (
        _bf(f32("wq").T), _bf(f32("wk").T), _bf(f32("wv").T), _bf(f32("wo").T)
    )
    wgT, wuT, wdT = _bf(f32("w_gate").T), _bf(f32("w_up").T), _bf(f32("w_down").T)
    ln1, ln2 = f32("ln1_w"), f32("ln2_w")

    kts, vs, xns, hts = {}, {}, {}, {}
    for core in range(NCORES):
        b, g = core // 4, core % 4
        ht = hidden[b][_core_tokens(g)].T  # [2048, 512]
        hts[core] = ht
        xn = _sim_norm(ht, ln1)
        xns[core] = xn
        kts[core] = _bf(wkT.T @ xn)  # kT [2048, 512]
        vs[core] = _bf(xn.T @ wvT)  # v natural [512, 2048]

    outs = []
    for core in range(NCORES):
        b, g = core // 4, core % 4
        grp = [4 * b + r for r in range(4)]
        kt_all = np.concatenate([kts[c] for c in grp], axis=1)  # [2048, 2048]
        v_all = np.concatenate([vs[c] for c in grp], axis=0)  # [2048, 2048]
        mask = np.asarray(_core_mask(g), dtype=np.float32)

        xn = xns[core]
        qT = _bf(wqT.T @ xn)  # [2048, 512]
        attn = np.zeros((D, Q), dtype=np.float32)
        for h in range(H):
            kth = kt_all[h * HD : (h + 1) * HD]  # [128, 2048]
            aps = np.zeros((HD, Q), dtype=np.float32)
            den = np.zeros(Q, dtype=np.float32)
            for jj in range(NB):
                off = _agcol(jj)
                n = Q if jj < 8 else BLK
                sc = kth[:, off : off + P].T @ qT[h * HD : (h + 1) * HD, Q - n :]
                e = _bf(_bf(np.exp(sc * ISQ)) * mask[:, jj, :n])
                vt = v_all[off : off + P, h * HD : (h + 1) * HD]  # [128, hd]
                aps[:, Q - n :] += vt.T @ e
                den[Q - n :] += e.sum(axis=0)
            attn[h * HD : (h + 1) * HD] = _bf(aps * (1.0 / den)[None, :])
        oT = woT.T @ attn  # fp32 accum of bf16 matmul
        h2 = hts[core] + oT
        yT = _sim_norm(h2, ln2)
        gate = wgT.T @ yT
        up = wuT.T @ yT
        sil = _bf(gate / (1.0 + np.exp(-gate)))
        hmid = _bf(sil * up)
        outT = h2 + wdT.T @ hmid
        outs.append(outT.reshape(KO, P, Q).transpose(1, 0, 2).astype(np.float32))
    return assemble(outs, np.asarray(inputs["hidden_states"]).dtype)


# ---------------------------------------------------------------------------
# Device program
# ---------------------------------------------------------------------------

def _build_bass():
    import concourse.bacc as bacc
    import concourse.mybir as mybir
    import concourse.tile as tile

    FP = mybir.dt.float32
    BF = mybir.dt.bfloat16
    AF = mybir.ActivationFunctionType

    nc = bacc.Bacc("TRN2", target_bir_lowering=False, debug=False,
                   num_devices=NCORES)

    ht_d = nc.dram_tensor("ht", [P, KO, Q], FP, kind="ExternalInput")
    mask_d = nc.dram_tensor("mask", [P, NB, Q], BF, kind="ExternalInput")
    ln1_d = nc.dram_tensor("ln1", [P, KO], FP, kind="ExternalInput")
    ln2_d = nc.dram_tensor("ln2", [P, KO], FP, kind="ExternalInput")
    wqt_d = nc.dram_tensor("wqt", [P, KO, KO, P], BF, kind="ExternalInput")
    wkt_d = nc.dram_tensor("wkt", [P, KO, KO, P], BF, kind="ExternalInput")
    wvt_d = nc.dram_tensor("wvt", [P, KO, D], BF, kind="ExternalInput")
    wot_d = nc.dram_tensor("wot", [P, KO, KO, P], BF, kind="ExternalInput")
    wgt_d = nc.dram_tensor("wgt", [P, MF, KO, P], BF, kind="ExternalInput")
    wut_d = nc.dram_tensor("wut", [P, MF, KO, P], BF, kind="ExternalInput")
    wdt_d = nc.dram_tensor("wdt", [P, KO, MF, P], BF, kind="ExternalInput")
    out_d = nc.dram_tensor("out", [P, KO, Q], FP, kind="ExternalOutput")

    groups = [[0, 1, 2, 3], [4, 5, 6, 7]]

    with tile.TileContext(nc) as tc, ExitStack() as top:
        dramp = top.enter_context(tc.tile_pool(name="dram", bufs=1, space="DRAM"))
        constp = top.enter_context(tc.tile_pool(name="const", bufs=1))
        statp = top.enter_context(tc.tile_pool(name="stat", bufs=2))
        workp = top.enter_context(tc.tile_pool(name="work", bufs=3))
        psump = top.enter_context(tc.tile_pool(name="ps", bufs=4, space="PSUM"))
        psaccp = psump

        ones = constp.tile([P, P], BF, tag="ones")
        nc.vector.memset(ones, 1.0)
        eps_t = constp.tile([P, 1], FP, tag="eps")
        nc.vector.memset(eps_t, EPS)
        lnw1 = constp.tile([P, KO], FP, tag="ln1")
        nc.sync.dma_start(lnw1, ln1_d[:])
        lnw2 = constp.tile([P, KO], FP, tag="ln2")
        nc.sync.dma_start(lnw2, ln2_d[:])

        ht_sb = constp.tile([P, KO, Q], FP, tag="ht")  # becomes h2 in place
        nc.sync.dma_start(ht_sb, ht_d[:])
        xn_sb = constp.tile([P, KO, Q], BF, tag="xn")  # x_norm^T, later y^T

        kv_bounce = [dramp.tile([2, D * Q // 2], BF, name=f"kvb{i}")
                     for i in range(2)]
        kv_all = [dramp.tile([8, D * Q // 2], BF, name=f"kva{i}")
                  for i in range(2)]
        kt_b = [b[0:1, :].rearrange("a (d q) -> (a d) q", q=Q)
                for b in kv_bounce]
        v_b = [b[1:2, :].rearrange("a (t d) -> (a t) d", d=D // 2)
               for b in kv_bounce]

        def norm(src, lnw, dst):
            ms = psaccp.tile([P, Q], FP, tag="acc")
            for ko in range(KO):
                sq = workp.tile([P, Q], BF, tag="sq")
                nc.vector.tensor_mul(sq, src[:, ko, :], src[:, ko, :])
                nc.tensor.matmul(ms, ones, sq, start=(ko == 0),
                                 stop=(ko == KO - 1))
            st = statp.tile([P, Q], FP, tag="st")
            nc.scalar.activation(st, ms, AF.Sqrt, bias=eps_t,
                                 scale=1.0 / D)
            rb = statp.tile([P, Q], FP, tag="rb")
            nc.vector.reciprocal(rb, st)
            for ko in range(KO):
                tmp = workp.tile([P, Q], FP, tag="nrm")
                nc.vector.tensor_mul(tmp, src[:, ko, :], rb)
                nc.vector.tensor_scalar_mul(dst[:, ko, :], tmp,
                                            lnw[:, ko : ko + 1])

        # ---- phase 1: norm1 ----
        norm(ht_sb, lnw1, xn_sb)

        with ExitStack() as mid:
            midp = mid.enter_context(tc.tile_pool(name="mid", bufs=1))
            qt_sb = midp.tile([P, KO, Q], BF, tag="qt")
            attn_sb = midp.tile([P, KO, Q], BF, tag="attn")
            with ExitStack() as ctx:
                wp = ctx.enter_context(tc.tile_pool(name="wqkv", bufs=3))
                wvp = ctx.enter_context(tc.tile_pool(name="wvp", bufs=2))

                def proj_t(w5_d, dst_fn, lo=0, hi=KO):
                    for mt in range(lo, hi):
                        wt = wp.tile([P, KO, P], BF, tag="wqk")
                        nc.sync.dma_start(wt, w5_d[:, mt, :, :])
                        ps = psump.tile([P, Q], FP, tag="mm")
                        for ko in range(KO):
                            nc.tensor.matmul(ps, wt[:, ko, :], xn_sb[:, ko, :],
                                             start=(ko == 0),
                                             stop=(ko == KO - 1))
                        dst_fn(mt, ps)

                # ---- phase 2: K projection + AG ----
                def k_out(mt, ps):
                    stg = workp.tile([P, Q], BF, tag="stg")
                    nc.vector.tensor_copy(stg, ps)
                    ml = mt % 8
                    nc.sync.dma_start(
                        kt_b[mt // 8][ml * P : (ml + 1) * P, :], stg
                    )

                # ---- phase 3: V projection, interleaved per-half AGs ----
                def v_chunk(n):
                    wv_c = wvp.tile([P, KO, Q], BF, tag="wv")
                    nc.sync.dma_start(wv_c, wvt_d[:, :, n * Q : (n + 1) * Q])
                    for tt in range(4):
                        ps = psump.tile([P, Q], FP, tag="mm")
                        for ko in range(KO):
                            nc.tensor.matmul(
                                ps, xn_sb[:, ko, tt * P : (tt + 1) * P],
                                wv_c[:, ko, :],
                                start=(ko == 0), stop=(ko == KO - 1),
                            )
                        stg = workp.tile([P, Q], BF, tag="stg")
                        nc.vector.tensor_copy(stg, ps)
                        nc.sync.dma_start(
                            v_b[n // 2][tt * P : (tt + 1) * P,
                                        (n % 2) * Q : (n % 2 + 1) * Q],
                            stg,
                        )

                for hh in range(2):
                    proj_t(wkt_d, k_out, 8 * hh, 8 * hh + 8)
                    v_chunk(2 * hh)
                    v_chunk(2 * hh + 1)
                    nc.gpsimd.collective_compute(
                        "AllGather", mybir.AluOpType.bypass,
                        ins=[kv_bounce[hh].opt()], outs=[kv_all[hh].opt()],
                        replica_groups=groups,
                    )

                # ---- phase 4: Q projection ----
                proj_t(wqt_d, lambda mt, ps:
                       nc.vector.tensor_copy(qt_sb[:, mt, :], ps))

            # ---- phase 5: attention ----
            with ExitStack() as ctx:
                ap = ctx.enter_context(tc.tile_pool(name="attp", bufs=3))
                eap = ctx.enter_context(tc.tile_pool(name="eap", bufs=2))
                maskp = ctx.enter_context(tc.tile_pool(name="maskp", bufs=1))
                mask_sb = maskp.tile([P, NB, Q], BF, tag="mask")
                nc.sync.dma_start(mask_sb, mask_d[:])
                kt_r = [kv_all[hh].rearrange(
                            "(r a) (p q) -> a r p q", a=2, q=Q
                        )[0].rearrange("r (m p) q -> p r m q", p=P)
                        for hh in range(2)]
                v_r = [kv_all[hh].rearrange(
                           "(r a) (t d) -> a r t d", a=2, d=D // 2
                       )[1] for hh in range(2)]

                for h in range(H):
                    hh, hl = h // 8, h % 8
                    kth = ap.tile([P, 4, Q], BF, tag="kth")
                    nc.sync.dma_start(kth, kt_r[hh][:, :, hl, :])
                    kth2 = kth.rearrange("p r q -> p (r q)")
                    vth = ap.tile([P, NB, HD], BF, tag="vth")
                    for jj in range(NB):
                        off = _agcol(jj)
                        nc.sync.dma_start(
                            vth[:, jj, :],
                            v_r[hh][off // Q, off % Q : off % Q + P,
                                    hl * HD : (hl + 1) * HD],
                        )
                    e_all = eap.tile([P, NB, Q], BF, tag="eall")
                    # pass A: scores + exp + mask (needs only KT)
                    for jj in range(NB):
                        n = Q if jj < 8 else BLK
                        off = _agcol(jj)
                        sps = psump.tile([P, Q], FP, tag="mm")
                        nc.tensor.matmul(
                            sps[:, :n], kth2[:, off : off + P],
                            qt_sb[:, h, Q - n :], start=True, stop=True,
                        )
                        nc.scalar.activation(e_all[:, jj, :n], sps[:, :n],
                                             AF.Exp, scale=ISQ)
                        nc.vector.tensor_mul(e_all[:, jj, :n],
                                             e_all[:, jj, :n],
                                             mask_sb[:, jj, :n])
                    # pass B: PV + denominator (needs V)
                    aps = psaccp.tile([P, Q], FP, tag="acc")
                    dps = psaccp.tile([P, Q], FP, tag="acc")
                    for jj in range(NB):
                        n = Q if jj < 8 else BLK
                        osl = slice(Q - n, Q)
                        nc.tensor.matmul(aps[:, osl], vth[:, jj, :],
                                         e_all[:, jj, :n],
                                         start=(jj == 0), stop=(jj == NB - 1))
                        nc.tensor.matmul(dps[:, osl], ones,
                                         e_all[:, jj, :n],
                                         start=(jj == 0), stop=(jj == NB - 1))
                    rec = statp.tile([P, Q], FP, tag="rb")
                    nc.vector.reciprocal(rec, dps)
                    nc.vector.tensor_mul(attn_sb[:, h, :], aps, rec)

            # ---- phase 6: o-projection + residual (into ht_sb) ----
            with ExitStack() as ctx:
                wp = ctx.enter_context(tc.tile_pool(name="wo", bufs=3))
                for mt in range(KO):
                    wt = wp.tile([P, KO, P], BF, tag="wqk")
                    nc.sync.dma_start(wt, wot_d[:, mt, :, :])
                    ps = psump.tile([P, Q], FP, tag="mm")
                    for ko in range(KO):
                        nc.tensor.matmul(ps, wt[:, ko, :], attn_sb[:, ko, :],
                                         start=(ko == 0), stop=(ko == KO - 1))
                    nc.vector.tensor_add(ht_sb[:, mt, :], ps, ht_sb[:, mt, :])

        # ---- phase 7: norm2 (into xn_sb = y^T) ----
        norm(ht_sb, lnw2, xn_sb)

        # ---- phases 8+9: MLP ----
        with ExitStack() as ctx:
            wgp = ctx.enter_context(tc.tile_pool(name="wgu", bufs=2))
            wdp = ctx.enter_context(tc.tile_pool(name="wdp", bufs=2))
            hp = ctx.enter_context(tc.tile_pool(name="hmid", bufs=1))
            hmid = hp.tile([P, MF, Q], BF, tag="hmid")
            for mf in range(MF):
                wg_t = wgp.tile([P, KO, P], BF, tag="wg")
                nc.sync.dma_start(wg_t, wgt_d[:, mf, :, :])
                wu_t = wgp.tile([P, KO, P], BF, tag="wu")
                nc.sync.dma_start(wu_t, wut_d[:, mf, :, :])
                gps = psump.tile([P, Q], FP, tag="mm")
                ups = psump.tile([P, Q], FP, tag="mm")
                for ko in range(KO):
                    nc.tensor.matmul(gps, wg_t[:, ko, :], xn_sb[:, ko, :],
                                     start=(ko == 0), stop=(ko == KO - 1))
                for ko in range(KO):
                    nc.tensor.matmul(ups, wu_t[:, ko, :], xn_sb[:, ko, :],
                                     start=(ko == 0), stop=(ko == KO - 1))
                sil = workp.tile([P, Q], BF, tag="sil")
                nc.scalar.activation(sil, gps, AF.Silu)
                nc.vector.tensor_mul(hmid[:, mf, :], sil, ups)

            for mt in range(KO):
                wd_t = wdp.tile([P, MF, P], BF, tag="wd")
                nc.sync.dma_start(wd_t, wdt_d[:, mt, :, :])
                ps = psump.tile([P, Q], FP, tag="mm")
                for kf in range(MF):
                    nc.tensor.matmul(ps, wd_t[:, kf, :], hmid[:, kf, :],
                                     start=(kf == 0), stop=(kf == MF - 1))
                ot = workp.tile([P, Q], FP, tag="ot")
                nc.vector.tensor_add(ot, ps, ht_sb[:, mt, :])
                nc.sync.dma_start(out_d[:, mt, :], ot)

    nc.compile()
    return nc


_NC_CACHE = None


def kernel(**inputs) -> np.ndarray:
    global _NC_CACHE
    hidden = np.asarray(inputs["hidden_states"])
    shared = prepare_shared(
        np.asarray(inputs["wq"]), np.asarray(inputs["wk"]),
        np.asarray(inputs["wv"]), np.asarray(inputs["wo"]),
        np.asarray(inputs["w_gate"]), np.asarray(inputs["w_up"]),
        np.asarray(inputs["w_down"]), np.asarray(inputs["ln1_w"]),
        np.asarray(inputs["ln2_w"]),
    )
    in_maps = []
    for core in range(NCORES):
        m = dict(shared)
        m.update(prepare_core(np.asarray(hidden, dtype=np.float32), core))
        in_maps.append(m)

    from concourse.bass_utils import run_bass_kernel_spmd

    if _NC_CACHE is None:
        _NC_CACHE = _build_bass()
    nc = _NC_CACHE
    trace = bool(int(os.environ.get("KERNEL_TRACE", "0")))
    res = run_bass_kernel_spmd(
        nc, in_maps, core_ids=list(range(NCORES)), trace=trace
    )
    if trace and res.exec_time_ns is not None:
        print(f"HW exec time: {res.exec_time_ns} ns")
    outs = [res.results[c]["out"] for c in range(NCORES)]
    return assemble(outs, hidden.dtype)



# revision 9
# speedup vs baseline: 1.1313x; 1.1313x over previous
"""BitNetV3 transformer block on 8 Trainium2 NeuronCores.

Sharding: sequence-parallel. Each core owns 512 query tokens (two
256-token blocks (g, g+4) of one batch element; cores 0-3 -> batch 0,
cores 4-7 -> batch 1). Weights are replicated and host-pre-transposed;
the attention path (q/k/v/o projections, scores, PV) runs in fp8-e4m3
with DoubleRow pairing on the contraction dimension, the MLP stays
bf16. K and V are exchanged with four 4-rank fp8 AllGathers (K half,
V half per 1024-d slice) pipelined behind the projection compute.
Causal masking uses host-supplied per-core 0/1 fp8 mask tiles so the
SPMD program is identical on every core. Activations live transposed
([d, token]); per-token reductions use ones-vector (DoubleRow) matmuls
onto all partitions.

Scale conventions (fp8 ranges): weights wq/wk/wv/wo are stored x32;
q/k/v circulate x32; e = exp(s*ISQ)/16 (fits fp8 max 240); attention
probabilities circulate x8; o-projection PSUM is x256 and is unscaled
by a ScalarE copy before the residual add.
"""

import os
from contextlib import ExitStack

import numpy as np
import ml_dtypes

# ---- problem constants (hardcoded per the harness contract) ----
B = 2
S = 2048
D = 2048
H = 16
HD = 128
DFF = 8192
EPS = 1e-6
ISQ = float(1.0 / np.sqrt(HD))

P = 128  # partitions
KO = D // P  # 16 d-tiles
Q = 512  # tokens per core
NB = S // P  # 16 k-tiles per batch
MF = DFF // P  # 64 dff-tiles
BLK = 256  # token block
NCORES = 8

WS = 32.0  # fp8 weight scale for wq/wk/wv/wo
ESH = float(np.log(16.0))  # e = exp(s) / 16
ATS = 8.0  # attn prob scale in fp8

BF16 = ml_dtypes.bfloat16
F8 = ml_dtypes.float8_e4m3  # TRN FP8_EXP4 (bias 7, max 240)


# ---------------------------------------------------------------------------
# Host-side data preparation (sharding + layout)
# ---------------------------------------------------------------------------

def _w5(w_t: np.ndarray, kt: int, mt: int, dtype, scale=1.0) -> np.ndarray:
    """[K, M] (transposed weight, contraction-major) -> [128, mt, kt, 128]
    with W5[p, m, k, i] = w_t[k*128+p, m*128+i]."""
    K, M = w_t.shape
    assert K == kt * P and M == mt * P
    return np.ascontiguousarray(
        (w_t * scale).reshape(kt, P, mt, P).transpose(1, 2, 0, 3)
    ).astype(dtype)


def _core_tokens(g: int) -> np.ndarray:
    t1 = np.arange(BLK * g, BLK * (g + 1))
    t2 = np.arange(BLK * (g + 4), BLK * (g + 5))
    return np.concatenate([t1, t2])


def _core_mask(g: int) -> np.ndarray:
    """[128, NB, 512] fp8 causal 0/1 mask, jj = global k-tile index.

    jj < 8: cols 0..511 = all 512 owned q tokens.
    jj >= 8: cols 0..255 = q tokens of block g+4 (shifted layout)."""
    toks = _core_tokens(g)
    m = np.zeros((P, NB, Q), dtype=np.float32)
    for jj in range(NB):
        kk = 128 * jj + np.arange(P)
        if jj < 8:
            m[:, jj, :] = kk[:, None] <= toks[None, :]
        else:
            m[:, jj, :BLK] = kk[:, None] <= toks[None, BLK:]
    return m.astype(F8)


def prepare_shared(wq, wk, wv, wo, w_gate, w_up, w_down, ln1_w, ln2_w):
    d = {}
    d["wqt"] = _w5(np.ascontiguousarray(wq.T), KO, KO, F8, WS)
    d["wkt"] = _w5(np.ascontiguousarray(wk.T), KO, KO, F8, WS)
    d["wot"] = _w5(np.ascontiguousarray(wo.T), KO, KO, F8, WS)
    # V projection rhs layout: [128, ko, 2048]
    d["wvt"] = np.ascontiguousarray(
        (wv.T * WS).reshape(KO, P, D).transpose(1, 0, 2)
    ).astype(F8)
    d["wgt"] = _w5(np.ascontiguousarray(w_gate.T), KO, MF, BF16)
    d["wut"] = _w5(np.ascontiguousarray(w_up.T), KO, MF, BF16)
    d["wdt"] = _w5(np.ascontiguousarray(w_down.T), MF, KO, BF16)
    d["ln1"] = np.ascontiguousarray(ln1_w.reshape(KO, P).T).astype(np.float32)
    d["ln2"] = np.ascontiguousarray(ln2_w.reshape(KO, P).T).astype(np.float32)
    return d


def prepare_core(hidden, core: int):
    b, g = core // 4, core % 4
    toks = _core_tokens(g)
    ht = hidden[b][toks].T  # [2048 d, 512 q]
    ht5 = np.ascontiguousarray(ht.reshape(KO, P, Q).transpose(1, 0, 2)).astype(
        np.float32
    )
    return {"ht": ht5, "mask": _core_mask(g)}


def assemble(outs, hidden_dtype):
    full = np.empty((B, S, D), dtype=np.float32)
    for core in range(NCORES):
        b, g = core // 4, core % 4
        toks = _core_tokens(g)
        o = np.asarray(outs[core])  # [p, ko, q]
        full[b, toks, :] = o.transpose(2, 1, 0).reshape(Q, D)
    return full.astype(hidden_dtype)


def _kslice(jj: int):
    """Global k-tile jj -> (rank r, col offset) in kt_all[r, ml, p, q]-style
    gathered K buffer (rank-major: rank r holds token blocks r and r+4)."""
    b, s = jj // 2, jj % 2
    return b % 4, 256 * (b // 4) + 128 * s


# ---------------------------------------------------------------------------
# Pure-numpy simulation of the exact device dataflow (for fast validation)
# ---------------------------------------------------------------------------

def _bf(x):
    return x.astype(BF16).astype(np.float32)


def _f8(x):
    return np.clip(np.asarray(x, np.float32), -240.0, 240.0).astype(F8).astype(
        np.float32
    )


def _sim_norm(ht, lnw, f8out):
    sq = _f8(ht * ht)
    ms = sq.sum(axis=0)
    rstd = 1.0 / np.sqrt(ms / D + EPS)
    o = (ht * rstd[None, :]) * lnw[:, None]
    return _f8(o) if f8out else _bf(o)


def host_simulate(inputs):
    hidden = np.asarray(inputs["hidden_states"], dtype=np.float32)
    f32 = lambda k: np.asarray(inputs[k], dtype=np.float32)  # noqa: E731
    wqT = _f8(f32("wq").T * WS)
    wkT = _f8(f32("wk").T * WS)
    wvT = _f8(f32("wv").T * WS)
    woT = _f8(f32("wo").T * WS)
    wgT, wuT, wdT = _bf(f32("w_gate").T), _bf(f32("w_up").T), _bf(f32("w_down").T)
    ln1, ln2 = f32("ln1_w"), f32("ln2_w")

    kts, vs, xns, hts = {}, {}, {}, {}
    for core in range(NCORES):
        b, g = core // 4, core % 4
        ht = hidden[b][_core_tokens(g)].T  # [2048, 512]
        hts[core] = ht
        xn = _sim_norm(ht, ln1, f8out=True)
        xns[core] = xn
        kts[core] = _f8(wkT.T @ xn)  # 32*kT [2048, 512]
        vs[core] = _f8(xn.T @ wvT)  # 32*v natural [512, 2048]

    outs = []
    for core in range(NCORES):
        b, g = core // 4, core % 4
        grp = [4 * b + r for r in range(4)]
        kt_all = np.stack([kts[c] for c in grp])  # [4, 2048, 512]
        v_all = np.stack([vs[c] for c in grp])  # [4, 512, 2048]
        mask = np.asarray(_core_mask(g), dtype=np.float32)

        xn = xns[core]
        qT = _f8(wqT.T @ xn)  # 32*qT [2048, 512]
        attn8 = np.zeros((D, Q), dtype=np.float32)
        for h in range(H):
            aps = np.zeros((HD, Q), dtype=np.float32)
            den = np.zeros(Q, dtype=np.float32)
            for jj in range(NB):
                r, off = _kslice(jj)
                n = Q if jj < 8 else BLK
                kth = kt_all[r, h * HD:(h + 1) * HD, off:off + P]  # [hd,128]
                sc = kth.T @ qT[h * HD:(h + 1) * HD, Q - n:]
                e = _f8(np.exp(sc * (ISQ / (WS * WS)) - ESH) * mask[:, jj, :n])
                vt = v_all[r, off:off + P, h * HD:(h + 1) * HD]  # [128, hd]
                aps[:, Q - n:] += vt.T @ e
                den[Q - n:] += e.sum(axis=0)
            rec = 1.0 / (4.0 * den)
            attn8[h * HD:(h + 1) * HD] = _f8(aps * rec[None, :])
        ops = woT.T @ attn8  # 256*o
        h2 = hts[core] + ops / (WS * ATS)
        yT = _sim_norm(h2, ln2, f8out=False)
        gate = wgT.T @ yT
        up = wuT.T @ yT
        sil = _bf(gate / (1.0 + np.exp(-gate)))
        hmid = _bf(sil * up)
        outT = h2 + wdT.T @ hmid
        outs.append(outT.reshape(KO, P, Q).transpose(1, 0, 2).astype(np.float32))
    return assemble(outs, np.asarray(inputs["hidden_states"]).dtype)


# ---------------------------------------------------------------------------
# Device program
# ---------------------------------------------------------------------------

def _build_bass():
    import concourse.bacc as bacc
    import concourse.mybir as mybir
    import concourse.tile as tile

    FP = mybir.dt.float32
    BF = mybir.dt.bfloat16
    F8D = mybir.dt.float8e4
    AF = mybir.ActivationFunctionType
    DR = mybir.MatmulPerfMode.DoubleRow

    nc = bacc.Bacc("TRN2", target_bir_lowering=False, debug=False,
                   num_devices=NCORES)

    ht_d = nc.dram_tensor("ht", [P, KO, Q], FP, kind="ExternalInput")
    mask_d = nc.dram_tensor("mask", [P, NB, Q], F8D, kind="ExternalInput")
    ln1_d = nc.dram_tensor("ln1", [P, KO], FP, kind="ExternalInput")
    ln2_d = nc.dram_tensor("ln2", [P, KO], FP, kind="ExternalInput")
    wqt_d = nc.dram_tensor("wqt", [P, KO, KO, P], F8D, kind="ExternalInput")
    wkt_d = nc.dram_tensor("wkt", [P, KO, KO, P], F8D, kind="ExternalInput")
    wvt_d = nc.dram_tensor("wvt", [P, KO, D], F8D, kind="ExternalInput")
    wot_d = nc.dram_tensor("wot", [P, KO, KO, P], F8D, kind="ExternalInput")
    wgt_d = nc.dram_tensor("wgt", [P, MF, KO, P], BF, kind="ExternalInput")
    wut_d = nc.dram_tensor("wut", [P, MF, KO, P], BF, kind="ExternalInput")
    wdt_d = nc.dram_tensor("wdt", [P, KO, MF, P], BF, kind="ExternalInput")
    out_d = nc.dram_tensor("out", [P, KO, Q], FP, kind="ExternalOutput")

    groups = [[0, 1, 2, 3], [4, 5, 6, 7]]

    with tile.TileContext(nc) as tc, ExitStack() as top:
        dramp = top.enter_context(tc.tile_pool(name="dram", bufs=1, space="DRAM"))
        constp = top.enter_context(tc.tile_pool(name="const", bufs=1))
        statp = top.enter_context(tc.tile_pool(name="stat", bufs=2))
        workp = top.enter_context(tc.tile_pool(name="work", bufs=3))
        psump = top.enter_context(tc.tile_pool(name="ps", bufs=4, space="PSUM"))

        # value 4.0: folds the x4 denominator unscale into the ones-matmul
        # (attn8 = aps / (4*den')); norm() compensates via its Sqrt scale.
        ones8 = constp.tile([P, 2, P], F8D, tag="ones8")
        nc.vector.memset(ones8, 4.0)
        eps_t = constp.tile([P, 1], FP, tag="eps")
        nc.vector.memset(eps_t, EPS)
        nln16 = constp.tile([P, 1], FP, tag="nln16")
        nc.vector.memset(nln16, -ESH)
        lnw1 = constp.tile([P, KO], FP, tag="ln1")
        nc.sync.dma_start(lnw1, ln1_d[:])
        lnw2 = constp.tile([P, KO], FP, tag="ln2")
        nc.sync.dma_start(lnw2, ln2_d[:])

        ht_sb = constp.tile([P, KO, Q], FP, tag="ht")  # becomes h2 in place
        for c in range(4):
            nc.sync.dma_start(ht_sb[:, 4 * c:4 * c + 4, :],
                              ht_d[:, 4 * c:4 * c + 4, :])
        xn8 = constp.tile([P, KO, Q], F8D, tag="xn8")  # x_norm^T fp8

        # k bounce/gathered: [ml*128+p, q] per half; v: [tok, 1024 d] per half
        kb = [dramp.tile([8 * P, Q], F8D, name=f"kb{i}") for i in range(2)]
        ka = [dramp.tile([4, 8 * P, Q], F8D, name=f"ka{i}") for i in range(2)]
        vb = [dramp.tile([Q, D // 2], F8D, name=f"vb{i}") for i in range(2)]
        va = [dramp.tile([4, Q, D // 2], F8D, name=f"va{i}") for i in range(2)]

        def norm(src, lnw, dst):
            # mean-square via fp8 DoubleRow ones-matmuls
            ms = psump.tile([P, Q], FP, tag="acc")
            for j in range(KO // 2):
                sq = workp.tile([P, 2, Q], F8D, tag="sq")
                for s in range(2):
                    nc.vector.tensor_mul(sq[:, s, :], src[:, 2 * j + s, :],
                                         src[:, 2 * j + s, :])
                nc.tensor.matmul(ms, ones8, sq, start=(j == 0),
                                 stop=(j == KO // 2 - 1), perf_mode=DR)
            st = statp.tile([P, Q], FP, tag="st")
            nc.scalar.activation(st, ms, AF.Sqrt, bias=eps_t,
                                 scale=1.0 / (4.0 * D))
            rb = statp.tile([P, Q], FP, tag="rb")
            nc.vector.reciprocal(rb, st)
            for ko in range(KO):
                tmp = workp.tile([P, Q], FP, tag="nrm")
                nc.vector.tensor_mul(tmp, src[:, ko, :], rb)
                nc.vector.tensor_scalar_mul(dst[:, ko, :], tmp,
                                            lnw[:, ko:ko + 1])

        # ---- phase 1: norm1 ----
        norm(ht_sb, lnw1, xn8)

        with ExitStack() as mid:
            midp = mid.enter_context(tc.tile_pool(name="mid", bufs=1))
            qt8 = midp.tile([P, KO, Q], F8D, tag="qt8")
            attn8 = midp.tile([P, KO, Q], F8D, tag="attn8")
            with ExitStack() as ctx:
                wp = ctx.enter_context(tc.tile_pool(name="wqkv", bufs=3))
                wvp = ctx.enter_context(tc.tile_pool(name="wvp", bufs=2))

                def proj_t(w5_d, dst_fn, lo=0, hi=KO):
                    for mt in range(lo, hi):
                        wt = wp.tile([P, KO, P], F8D, tag="wqk")
                        nc.sync.dma_start(wt, w5_d[:, mt, :, :])
                        ps = psump.tile([P, Q], FP, tag="mm")
                        for j in range(KO // 2):
                            nc.tensor.matmul(
                                ps, wt[:, 2 * j:2 * j + 2, :],
                                xn8[:, 2 * j:2 * j + 2, :],
                                start=(j == 0), stop=(j == KO // 2 - 1),
                                perf_mode=DR,
                            )
                        dst_fn(mt, ps)

                def k_out(mt, ps):
                    stg = workp.tile([P, Q], F8D, tag="stg")
                    nc.vector.tensor_copy(stg, ps)
                    ml = mt % 8
                    nc.sync.dma_start(kb[mt // 8][ml * P:(ml + 1) * P, :], stg)

                def v_chunk(n):
                    wv_c = wvp.tile([P, KO, Q], F8D, tag="wv")
                    nc.sync.dma_start(wv_c, wvt_d[:, :, n * Q:(n + 1) * Q])
                    for tt in range(4):
                        ps = psump.tile([P, Q], FP, tag="mm")
                        for j in range(KO // 2):
                            nc.tensor.matmul(
                                ps,
                                xn8[:, 2 * j:2 * j + 2, tt * P:(tt + 1) * P],
                                wv_c[:, 2 * j:2 * j + 2, :],
                                start=(j == 0), stop=(j == KO // 2 - 1),
                                perf_mode=DR,
                            )
                        stg = workp.tile([P, Q], F8D, tag="stg")
                        nc.vector.tensor_copy(stg, ps)
                        nc.sync.dma_start(
                            vb[n // 2][tt * P:(tt + 1) * P,
                                       (n % 2) * Q:(n % 2 + 1) * Q],
                            stg,
                        )

                for hh in range(2):
                    proj_t(wkt_d, k_out, 8 * hh, 8 * hh + 8)
                    nc.gpsimd.collective_compute(
                        "AllGather", mybir.AluOpType.bypass,
                        ins=[kb[hh].opt()],
                        outs=[ka[hh].rearrange("r m q -> (r m) q").opt()],
                        replica_groups=groups,
                    )
                    v_chunk(2 * hh)
                    v_chunk(2 * hh + 1)
                    nc.gpsimd.collective_compute(
                        "AllGather", mybir.AluOpType.bypass,
                        ins=[vb[hh].opt()],
                        outs=[va[hh].rearrange("r t d -> (r t) d").opt()],
                        replica_groups=groups,
                    )

                # ---- phase 4: Q projection ----
                proj_t(wqt_d, lambda mt, ps:
                       nc.vector.tensor_copy(qt8[:, mt, :], ps))

            # ---- phase 5: attention ----
            with ExitStack() as ctx:
                kvp = ctx.enter_context(tc.tile_pool(name="kvp", bufs=2))
                eap = ctx.enter_context(tc.tile_pool(name="eap", bufs=2))
                maskp = ctx.enter_context(tc.tile_pool(name="maskp", bufs=1))
                mask_sb = maskp.tile([P, NB, Q], F8D, tag="mask")
                nc.sync.dma_start(mask_sb, mask_d[:])
                kt_sb = {}
                v_sb = {}
                for hh in range(2):
                    # kt_sb: [p, r, ml, q]   (ml = head-within-half)
                    kt_sb[hh] = kvp.tile([P, 4, 8, Q], F8D, tag="ktsb",
                                         name=f"ktsb{hh}")
                    nc.sync.dma_start(
                        kt_sb[hh],
                        ka[hh].rearrange("r (m p) q -> p r m q", p=P),
                    )
                    # v_sb: [p, jj, d-in-half]; jj = 8u+2r+s from row
                    # 256u+128s+p of rank r  (4 DMAs: dims must stay <=3)
                    v_sb[hh] = kvp.tile([P, NB, D // 2], F8D, tag="vsb",
                                        name=f"vsb{hh}")
                    vdst = v_sb[hh].rearrange("p (u r s) d -> p u r s d",
                                              u=2, r=4)
                    for u in range(2):
                        for s2 in range(2):
                            row0 = 256 * u + 128 * s2
                            nc.sync.dma_start(
                                vdst[:, u, :, s2, :],
                                va[hh][:, row0:row0 + P, :].rearrange(
                                    "r p d -> p r d"),
                            )

                for h in range(H):
                    hh, hl = h // 8, h % 8
                    e8 = eap.tile([P, NB, Q], F8D, tag="e8")
                    # pass A: scores + exp + mask
                    for jj in range(NB):
                        n = Q if jj < 8 else BLK
                        r, off = _kslice(jj)
                        sps = psump.tile([P, Q], FP, tag="mm")
                        nc.tensor.matmul(
                            sps[:, :n],
                            kt_sb[hh][:, r, hl, off:off + P],
                            qt8[:, h, Q - n:], start=True, stop=True,
                        )
                        ebf = workp.tile([P, Q], BF, tag="ebf")
                        nc.scalar.activation(ebf[:, :n], sps[:, :n], AF.Exp,
                                             bias=nln16,
                                             scale=ISQ / (WS * WS))
                        nc.vector.tensor_mul(e8[:, jj, :n], ebf[:, :n],
                                             mask_sb[:, jj, :n])
                    # pass B: PV + denominator via fp8 DoubleRow
                    aps = psump.tile([P, Q], FP, tag="acc")
                    dps = psump.tile([P, Q], FP, tag="acc")
                    for j in range(NB // 2):
                        jj = 2 * j
                        n = Q if jj < 8 else BLK
                        osl = slice(Q - n, Q)
                        b2 = jj // 2
                        r, row0 = b2 % 4, 256 * (b2 // 4)
                        nc.tensor.matmul(
                            aps[:, osl],
                            v_sb[hh][:, jj:jj + 2,
                                     hl * HD:(hl + 1) * HD],
                            e8[:, jj:jj + 2, :n],
                            start=(j == 0), stop=(j == NB // 2 - 1),
                            perf_mode=DR,
                        )
                        nc.tensor.matmul(
                            dps[:, osl], ones8, e8[:, jj:jj + 2, :n],
                            start=(j == 0), stop=(j == NB // 2 - 1),
                            perf_mode=DR,
                        )
                    rec = statp.tile([P, Q], FP, tag="rb")
                    nc.vector.reciprocal(rec, dps)
                    nc.vector.tensor_mul(attn8[:, h, :], aps, rec)

            # ---- phase 6: o-projection + residual (into ht_sb) ----
            with ExitStack() as ctx:
                wp = ctx.enter_context(tc.tile_pool(name="wo", bufs=3))
                for mt in range(KO):
                    wt = wp.tile([P, KO, P], F8D, tag="wqk")
                    nc.sync.dma_start(wt, wot_d[:, mt, :, :])
                    ps = psump.tile([P, Q], FP, tag="mm")
                    for j in range(KO // 2):
                        nc.tensor.matmul(
                            ps, wt[:, 2 * j:2 * j + 2, :],
                            attn8[:, 2 * j:2 * j + 2, :],
                            start=(j == 0), stop=(j == KO // 2 - 1),
                            perf_mode=DR,
                        )
                    ot = workp.tile([P, Q], FP, tag="ot")
                    nc.scalar.activation(ot, ps, AF.Copy,
                                         scale=1.0 / (WS * ATS))
                    nc.vector.tensor_add(ht_sb[:, mt, :], ot, ht_sb[:, mt, :])

        # ---- phase 7: norm2 (bf16 out for the MLP) ----
        with ExitStack() as ctx:
            ynp = ctx.enter_context(tc.tile_pool(name="yn", bufs=1))
            ynb = ynp.tile([P, KO, Q], BF, tag="ynb")
            norm(ht_sb, lnw2, ynb)

            # ---- phases 8+9: MLP (bf16) ----
            with ExitStack() as ctx2:
                wgp = ctx2.enter_context(tc.tile_pool(name="wgu", bufs=2))
                wdp = ctx2.enter_context(tc.tile_pool(name="wdp", bufs=2))
                hp = ctx2.enter_context(tc.tile_pool(name="hmid", bufs=1))
                hmid = hp.tile([P, MF, Q], BF, tag="hmid")
                for mf in range(MF):
                    wg_t = wgp.tile([P, KO, P], BF, tag="wg")
                    nc.sync.dma_start(wg_t, wgt_d[:, mf, :, :])
                    wu_t = wgp.tile([P, KO, P], BF, tag="wu")
                    nc.sync.dma_start(wu_t, wut_d[:, mf, :, :])
                    gps = psump.tile([P, Q], FP, tag="mm")
                    ups = psump.tile([P, Q], FP, tag="mm")
                    for ko in range(KO):
                        nc.tensor.matmul(gps, wg_t[:, ko, :], ynb[:, ko, :],
                                         start=(ko == 0), stop=(ko == KO - 1))
                    for ko in range(KO):
                        nc.tensor.matmul(ups, wu_t[:, ko, :], ynb[:, ko, :],
                                         start=(ko == 0), stop=(ko == KO - 1))
                    sil = workp.tile([P, Q], BF, tag="sil")
                    nc.scalar.activation(sil, gps, AF.Silu)
                    nc.vector.tensor_mul(hmid[:, mf, :], sil, ups)

                for mt in range(KO):
                    wd_t = wdp.tile([P, MF, P], BF, tag="wd")
                    nc.sync.dma_start(wd_t, wdt_d[:, mt, :, :])
                    ps = psump.tile([P, Q], FP, tag="mm")
                    for kf in range(MF):
                        nc.tensor.matmul(ps, wd_t[:, kf, :], hmid[:, kf, :],
                                         start=(kf == 0), stop=(kf == MF - 1))
                    ot = workp.tile([P, Q], FP, tag="ot")
                    nc.vector.tensor_add(ot, ps, ht_sb[:, mt, :])
                    nc.sync.dma_start(out_d[:, mt, :], ot)

    nc.compile()
    return nc


_NC_CACHE = None


def kernel(**inputs) -> np.ndarray:
    global _NC_CACHE
    hidden = np.asarray(inputs["hidden_states"])
    shared = prepare_shared(
        np.asarray(inputs["wq"]), np.asarray(inputs["wk"]),
        np.asarray(inputs["wv"]), np.asarray(inputs["wo"]),
        np.asarray(inputs["w_gate"]), np.asarray(inputs["w_up"]),
        np.asarray(inputs["w_down"]), np.asarray(inputs["ln1_w"]),
        np.asarray(inputs["ln2_w"]),
    )
    in_maps = []
    for core in range(NCORES):
        m = dict(shared)
        m.update(prepare_core(np.asarray(hidden, dtype=np.float32), core))
        in_maps.append(m)

    from concourse.bass_utils import run_bass_kernel_spmd

    if _NC_CACHE is None:
        _NC_CACHE = _build_bass()
    nc = _NC_CACHE
    trace = bool(int(os.environ.get("KERNEL_TRACE", "0")))
    res = run_bass_kernel_spmd(
        nc, in_maps, core_ids=list(range(NCORES)), trace=trace
    )
    if trace and res.exec_time_ns is not None:
        print(f"HW exec time: {res.exec_time_ns} ns")
    outs = [res.results[c]["out"] for c in range(NCORES)]
    return assemble(outs, hidden.dtype)


# revision 18
# speedup vs baseline: 1.1959x; 1.0571x over previous
"""BitNetV3 transformer block on 8 Trainium2 NeuronCores.

Sharding: sequence-parallel. Each core owns 512 query tokens (two
256-token blocks (g, g+4) of one batch element; cores 0-3 -> batch 0,
cores 4-7 -> batch 1). Weights are replicated and host-pre-transposed;
the attention path (q/k/v/o projections, scores, PV) runs in fp8-e4m3
with DoubleRow pairing on the contraction dimension, the MLP stays
bf16. K and V are exchanged with four 4-rank fp8 AllGathers (K half,
V half per 1024-d slice) pipelined behind the projection compute.
Causal masking uses host-supplied per-core 0/1 fp8 mask tiles so the
SPMD program is identical on every core. Activations live transposed
([d, token]); per-token reductions use ones-vector (DoubleRow) matmuls
onto all partitions.

Scale conventions (fp8 ranges): weights wq/wk/wv/wo are stored x32;
q/k/v circulate x32; e = exp(s*ISQ)/16 (fits fp8 max 240); attention
probabilities circulate x8; o-projection PSUM is x256 and is unscaled
by a ScalarE copy before the residual add.
"""

import os
from contextlib import ExitStack

import numpy as np
import ml_dtypes

# ---- problem constants (hardcoded per the harness contract) ----
B = 2
S = 2048
D = 2048
H = 16
HD = 128
DFF = 8192
EPS = 1e-6
ISQ = float(1.0 / np.sqrt(HD))

P = 128  # partitions
KO = D // P  # 16 d-tiles
Q = 512  # tokens per core
NB = S // P  # 16 k-tiles per batch
MF = DFF // P  # 64 dff-tiles
BLK = 256  # token block
NCORES = 8

WS = 32.0  # fp8 weight scale for wq/wk/wv/wo
ESH = float(np.log(16.0))  # e = exp(s) / 16
ATS = 8.0  # attn prob scale in fp8

BF16 = ml_dtypes.bfloat16
F8 = ml_dtypes.float8_e4m3  # TRN FP8_EXP4 (bias 7, max 240)


# ---------------------------------------------------------------------------
# Host-side data preparation (sharding + layout)
# ---------------------------------------------------------------------------

def _w5(w_t: np.ndarray, kt: int, mt: int, dtype, scale=1.0) -> np.ndarray:
    """[K, M] (transposed weight, contraction-major) -> [128, mt, kt, 128]
    with W5[p, m, k, i] = w_t[k*128+p, m*128+i]."""
    K, M = w_t.shape
    assert K == kt * P and M == mt * P
    return np.ascontiguousarray(
        (w_t * scale).reshape(kt, P, mt, P).transpose(1, 2, 0, 3)
    ).astype(dtype)


def _core_tokens(g: int) -> np.ndarray:
    t1 = np.arange(BLK * g, BLK * (g + 1))
    t2 = np.arange(BLK * (g + 4), BLK * (g + 5))
    return np.concatenate([t1, t2])


def _core_mask(g: int) -> np.ndarray:
    """[128, NB, 256] fp8 causal 0/1 mask for the maskable region only.

    jj < 8: cols = q tokens of block g (device cols 0..255; cols 256..511,
    block g+4, are always causally visible for k-blocks 0..3).
    jj >= 8: cols = q tokens of block g+4 (shifted layout)."""
    toks = _core_tokens(g)
    m = np.zeros((P, NB, BLK), dtype=np.float32)
    for jj in range(NB):
        kk = 128 * jj + np.arange(P)
        if jj < 8:
            m[:, jj, :] = kk[:, None] <= toks[None, :BLK]
        else:
            m[:, jj, :] = kk[:, None] <= toks[None, BLK:]
    return m.astype(F8)


def prepare_shared(wq, wk, wv, wo, w_gate, w_up, w_down, ln1_w, ln2_w):
    d = {}
    d["wqt"] = _w5(np.ascontiguousarray(wq.T), KO, KO, F8, WS)
    d["wkt"] = _w5(np.ascontiguousarray(wk.T), KO, KO, F8, WS)
    d["wot"] = _w5(np.ascontiguousarray(wo.T), KO, KO, F8, WS)
    # V projection rhs layout: [128, ko, 2048]
    d["wvt"] = np.ascontiguousarray(
        (wv.T * WS).reshape(KO, P, D).transpose(1, 0, 2)
    ).astype(F8)
    d["wgt"] = _w5(np.ascontiguousarray(w_gate.T), KO, MF, BF16)
    d["wut"] = _w5(np.ascontiguousarray(w_up.T), KO, MF, BF16)
    d["wdt"] = _w5(np.ascontiguousarray(w_down.T), MF, KO, BF16)
    d["ln1"] = np.ascontiguousarray(ln1_w.reshape(KO, P).T).astype(np.float32)
    d["ln2"] = np.ascontiguousarray(ln2_w.reshape(KO, P).T).astype(np.float32)
    return d


def prepare_core(hidden, core: int):
    b, g = core // 4, core % 4
    toks = _core_tokens(g)
    ht = hidden[b][toks].T  # [2048 d, 512 q]
    ht5 = np.ascontiguousarray(ht.reshape(KO, P, Q).transpose(1, 0, 2)).astype(
        np.float32
    )
    return {"ht": ht5, "mask": _core_mask(g)}


def assemble(outs, hidden_dtype):
    full = np.empty((B, S, D), dtype=np.float32)
    for core in range(NCORES):
        b, g = core // 4, core % 4
        toks = _core_tokens(g)
        o = np.asarray(outs[core])  # [p, ko, q]
        full[b, toks, :] = o.transpose(2, 1, 0).reshape(Q, D)
    return full.astype(hidden_dtype)


def _kslice(jj: int):
    """Global k-tile jj -> (rank r, col offset) in kt_all[r, ml, p, q]-style
    gathered K buffer (rank-major: rank r holds token blocks r and r+4)."""
    b, s = jj // 2, jj % 2
    return b % 4, 256 * (b // 4) + 128 * s


# ---------------------------------------------------------------------------
# Pure-numpy simulation of the exact device dataflow (for fast validation)
# ---------------------------------------------------------------------------

def _bf(x):
    return x.astype(BF16).astype(np.float32)


def _f8(x):
    return np.clip(np.asarray(x, np.float32), -240.0, 240.0).astype(F8).astype(
        np.float32
    )


def _sim_norm(ht, lnw, f8out):
    sq = _f8(ht * ht)
    ms = sq.sum(axis=0)
    rstd = 1.0 / np.sqrt(ms / D + EPS)
    o = (ht * rstd[None, :]) * lnw[:, None]
    return _f8(o) if f8out else _bf(o)


def host_simulate(inputs):
    hidden = np.asarray(inputs["hidden_states"], dtype=np.float32)
    f32 = lambda k: np.asarray(inputs[k], dtype=np.float32)  # noqa: E731
    wqT = _f8(f32("wq").T * WS)
    wkT = _f8(f32("wk").T * WS)
    wvT = _f8(f32("wv").T * WS)
    woT = _f8(f32("wo").T * WS)
    wgT, wuT, wdT = _bf(f32("w_gate").T), _bf(f32("w_up").T), _bf(f32("w_down").T)
    ln1, ln2 = f32("ln1_w"), f32("ln2_w")

    kts, vs, xns, hts = {}, {}, {}, {}
    for core in range(NCORES):
        b, g = core // 4, core % 4
        ht = hidden[b][_core_tokens(g)].T  # [2048, 512]
        hts[core] = ht
        xn = _sim_norm(ht, ln1, f8out=True)
        xns[core] = xn
        kts[core] = _f8(wkT.T @ xn)  # 32*kT [2048, 512]
        vs[core] = _f8(xn.T @ wvT)  # 32*v natural [512, 2048]

    outs = []
    for core in range(NCORES):
        b, g = core // 4, core % 4
        grp = [4 * b + r for r in range(4)]
        kt_all = np.stack([kts[c] for c in grp])  # [4, 2048, 512]
        v_all = np.stack([vs[c] for c in grp])  # [4, 512, 2048]
        mask = np.asarray(_core_mask(g), dtype=np.float32)

        xn = xns[core]
        qT = _f8(wqT.T @ xn)  # 32*qT [2048, 512]
        attn8 = np.zeros((D, Q), dtype=np.float32)
        for h in range(H):
            aps = np.zeros((HD, Q), dtype=np.float32)
            den = np.zeros(Q, dtype=np.float32)
            for jj in range(NB):
                r, off = _kslice(jj)
                n = Q if jj < 8 else BLK
                kth = kt_all[r, h * HD:(h + 1) * HD, off:off + P]  # [hd,128]
                sc = kth.T @ qT[h * HD:(h + 1) * HD, Q - n:]
                ex = np.exp(sc * (ISQ / (WS * WS)) - ESH)
                e = np.concatenate(
                    [_f8(ex[:, :BLK] * mask[:, jj, :]), _f8(ex[:, BLK:])],
                    axis=1)
                vt = v_all[r, off:off + P, h * HD:(h + 1) * HD]  # [128, hd]
                aps[:, Q - n:] += vt.T @ e
                den[Q - n:] += e.sum(axis=0)
            rec = 1.0 / (4.0 * den)
            attn8[h * HD:(h + 1) * HD] = _f8(aps * rec[None, :])
        ops = woT.T @ attn8  # 256*o
        h2 = hts[core] + ops / (WS * ATS)
        yT = _sim_norm(h2, ln2, f8out=False)
        gate = wgT.T @ yT
        up = wuT.T @ yT
        sil = _bf(gate / (1.0 + np.exp(-gate)))
        hmid = _bf(sil * up)
        outT = h2 + wdT.T @ hmid
        outs.append(outT.reshape(KO, P, Q).transpose(1, 0, 2).astype(np.float32))
    return assemble(outs, np.asarray(inputs["hidden_states"]).dtype)


# ---------------------------------------------------------------------------
# Device program
# ---------------------------------------------------------------------------

def _build_bass():
    import concourse.bacc as bacc
    import concourse.mybir as mybir
    import concourse.tile as tile

    FP = mybir.dt.float32
    BF = mybir.dt.bfloat16
    F8D = mybir.dt.float8e4
    AF = mybir.ActivationFunctionType
    DR = mybir.MatmulPerfMode.DoubleRow

    nc = bacc.Bacc("TRN2", target_bir_lowering=False, debug=False,
                   num_devices=NCORES)

    ht_d = nc.dram_tensor("ht", [P, KO, Q], FP, kind="ExternalInput")
    mask_d = nc.dram_tensor("mask", [P, NB, BLK], F8D, kind="ExternalInput")
    ln1_d = nc.dram_tensor("ln1", [P, KO], FP, kind="ExternalInput")
    ln2_d = nc.dram_tensor("ln2", [P, KO], FP, kind="ExternalInput")
    wqt_d = nc.dram_tensor("wqt", [P, KO, KO, P], F8D, kind="ExternalInput")
    wkt_d = nc.dram_tensor("wkt", [P, KO, KO, P], F8D, kind="ExternalInput")
    wvt_d = nc.dram_tensor("wvt", [P, KO, D], F8D, kind="ExternalInput")
    wot_d = nc.dram_tensor("wot", [P, KO, KO, P], F8D, kind="ExternalInput")
    wgt_d = nc.dram_tensor("wgt", [P, MF, KO, P], BF, kind="ExternalInput")
    wut_d = nc.dram_tensor("wut", [P, MF, KO, P], BF, kind="ExternalInput")
    wdt_d = nc.dram_tensor("wdt", [P, KO, MF, P], BF, kind="ExternalInput")
    out_d = nc.dram_tensor("out", [P, KO, Q], FP, kind="ExternalOutput")

    groups = [[0, 1, 2, 3], [4, 5, 6, 7]]

    with tile.TileContext(nc) as tc, ExitStack() as top:
        dramp = top.enter_context(tc.tile_pool(name="dram", bufs=1, space="DRAM"))
        constp = top.enter_context(tc.tile_pool(name="const", bufs=1))
        statp = top.enter_context(tc.tile_pool(name="stat", bufs=2))
        workp = top.enter_context(tc.tile_pool(name="work", bufs=3))
        # 2 PSUM banks for long-lived accumulators (norm ms / attn aps+dps)
        psacc = top.enter_context(tc.tile_pool(name="psacc", bufs=2,
                                               space="PSUM"))

        # value 4.0: folds the x4 denominator unscale into the ones-matmul
        # (attn8 = aps / (4*den')); norm() compensates via its Sqrt scale.
        ones8 = constp.tile([P, 2, P], F8D, tag="ones8")
        nc.vector.memset(ones8, 4.0)
        eps_t = constp.tile([P, 1], FP, tag="eps")
        nc.vector.memset(eps_t, EPS)
        nln16 = constp.tile([P, 1], FP, tag="nln16")
        nc.vector.memset(nln16, -ESH)
        lnw1 = constp.tile([P, KO], FP, tag="ln1")
        nc.sync.dma_start(lnw1, ln1_d[:])
        lnw2 = constp.tile([P, KO], FP, tag="ln2")
        nc.sync.dma_start(lnw2, ln2_d[:])

        ht_sb = constp.tile([P, KO, Q], FP, tag="ht")  # becomes h2 in place
        for c in range(4):
            nc.sync.dma_start(ht_sb[:, 4 * c:4 * c + 4, :],
                              ht_d[:, 4 * c:4 * c + 4, :])
        # x_norm^T fp8, split per contraction pair so consumers can start
        # before the full norm completes
        xn8p = [constp.tile([P, 2, Q], F8D, tag=f"xn8_{j}", name=f"xn8_{j}")
                for j in range(KO // 2)]

        # k bounce/gathered: [ml*128+p, q] per half; v: [tok, 1024 d] per half
        kb = [dramp.tile([8 * P, Q], F8D, name=f"kb{i}") for i in range(2)]
        ka = [dramp.tile([4, 8 * P, Q], F8D, name=f"ka{i}") for i in range(2)]
        vb = [dramp.tile([Q, D // 2], F8D, name=f"vb{i}") for i in range(2)]
        va = [dramp.tile([4, Q, D // 2], F8D, name=f"va{i}") for i in range(2)]

        def norm(src, lnw, dst_fn):
            # mean-square via ScalarE squares + fp8 DoubleRow ones-matmuls;
            # normalize = DVE mul (x*rstd) + ScalarE per-partition ln scale
            ms = psacc.tile([P, Q], FP, tag="acc")
            for j in range(KO // 2):
                sq = workp.tile([P, 2, Q], F8D, tag="sq")
                nc.scalar.activation(sq, src[:, 2 * j:2 * j + 2, :], AF.Square)
                nc.tensor.matmul(ms, ones8, sq, start=(j == 0),
                                 stop=(j == KO // 2 - 1), perf_mode=DR)
            st = statp.tile([P, Q], FP, tag="st")
            nc.scalar.activation(st, ms, AF.Sqrt, bias=eps_t,
                                 scale=1.0 / (4.0 * D))
            rb = statp.tile([P, Q], FP, tag="rb")
            nc.vector.reciprocal(rb, st)
            for ko in range(KO):
                tmp = workp.tile([P, Q], FP, tag="nrm")
                nc.vector.tensor_mul(tmp, src[:, ko, :], rb)
                dst_fn(ko, tmp)

        # ---- phase 1: norm1 ----
        norm(ht_sb, lnw1,
             lambda ko, tmp: nc.scalar.activation(
                 xn8p[ko // 2][:, ko % 2, :], tmp, AF.Copy,
                 scale=lnw1[:, ko:ko + 1]))

        with ExitStack() as mid:
            midp = mid.enter_context(tc.tile_pool(name="mid", bufs=1))
            qt8 = midp.tile([P, KO, Q], F8D, tag="qt8")
            attn8 = midp.tile([P, KO, Q], F8D, tag="attn8")
            maskp = mid.enter_context(tc.tile_pool(name="maskp", bufs=1))
            mask_sb = maskp.tile([P, NB, BLK], F8D, tag="mask")
            nc.sync.dma_start(mask_sb, mask_d[:])
            with ExitStack() as ctx:
                wkp = ctx.enter_context(tc.tile_pool(name="wkp", bufs=16))
                wqp = ctx.enter_context(tc.tile_pool(name="wqp", bufs=16))
                wvp = ctx.enter_context(tc.tile_pool(name="wvp", bufs=4))
                pmm = ctx.enter_context(tc.tile_pool(name="pmm", bufs=4,
                                                     space="PSUM"))
                # prefetch ALL qkv weights up-front (DMA queues are idle
                # now; later they compete with the AllGathers)
                wk_t, wq_t, wv_t = [], [], []
                for mt in range(KO):
                    wt = wkp.tile([P, KO, P], F8D, tag="wk", name=f"wk{mt}")
                    nc.sync.dma_start(wt, wkt_d[:, mt, :, :])
                    wk_t.append(wt)
                for mt in range(KO):
                    wt = wqp.tile([P, KO, P], F8D, tag="wq", name=f"wq{mt}")
                    nc.sync.dma_start(wt, wqt_d[:, mt, :, :])
                    wq_t.append(wt)
                for n in range(4):
                    wv_c = wvp.tile([P, KO, Q], F8D, tag="wv", name=f"wv{n}")
                    nc.sync.dma_start(wv_c, wvt_d[:, :, n * Q:(n + 1) * Q])
                    wv_t.append(wv_c)

                def proj_t(w_tiles, dst_fn, lo=0, hi=KO):
                    for mt in range(lo, hi):
                        wt = w_tiles[mt]
                        ps = pmm.tile([P, Q], FP, tag="mm")
                        for j in range(KO // 2):
                            nc.tensor.matmul(
                                ps, wt[:, 2 * j:2 * j + 2, :], xn8p[j],
                                start=(j == 0), stop=(j == KO // 2 - 1),
                                perf_mode=DR,
                            )
                        dst_fn(mt, ps)

                def k_out(mt, ps):
                    stg = workp.tile([P, Q], F8D, tag="stg")
                    nc.vector.tensor_copy(stg, ps)
                    ml = mt % 8
                    nc.sync.dma_start(kb[mt // 8][ml * P:(ml + 1) * P, :], stg)

                def v_chunk(n):
                    wv_c = wv_t[n]
                    for tt in range(4):
                        ps = pmm.tile([P, Q], FP, tag="mm")
                        for j in range(KO // 2):
                            nc.tensor.matmul(
                                ps,
                                xn8p[j][:, :, tt * P:(tt + 1) * P],
                                wv_c[:, 2 * j:2 * j + 2, :],
                                start=(j == 0), stop=(j == KO // 2 - 1),
                                perf_mode=DR,
                            )
                        stg = workp.tile([P, Q], F8D, tag="stg")
                        nc.vector.tensor_copy(stg, ps)
                        nc.sync.dma_start(
                            vb[n // 2][tt * P:(tt + 1) * P,
                                       (n % 2) * Q:(n % 2 + 1) * Q],
                            stg,
                        )

                def ag(inb, outb, pat):
                    nc.gpsimd.collective_compute(
                        "AllGather", mybir.AluOpType.bypass,
                        ins=[inb.opt()],
                        outs=[outb.rearrange(pat).opt()],
                        replica_groups=groups,
                    )

                # pipeline: K0 -> AGk0 | Q | V0 -> AGv0 | K1 -> AGk1 |
                #           V1 -> AGv1  (attention h0-7 needs k0+v0 first)
                proj_t(wk_t, k_out, 0, 8)
                ag(kb[0], ka[0], "r m q -> (r m) q")
                proj_t(wq_t, lambda mt, ps:
                       nc.vector.tensor_copy(qt8[:, mt, :], ps))
                v_chunk(0)
                v_chunk(1)
                ag(vb[0], va[0], "r t d -> (r t) d")
                proj_t(wk_t, k_out, 8, 16)
                ag(kb[1], ka[1], "r m q -> (r m) q")
                v_chunk(2)
                v_chunk(3)
                ag(vb[1], va[1], "r t d -> (r t) d")

            # ---- phase 5: attention ----
            with ExitStack() as ctx:
                kvp = ctx.enter_context(tc.tile_pool(name="kvp", bufs=2))
                eap = ctx.enter_context(tc.tile_pool(name="eap", bufs=2))
                psA = ctx.enter_context(tc.tile_pool(name="psA", bufs=2,
                                                     space="PSUM"))
                psB = ctx.enter_context(tc.tile_pool(name="psB", bufs=2,
                                                     space="PSUM"))
                kt_sb = {}
                v_sb = {}
                for hh in range(2):
                    # kt_sb: [p, r, ml, q]   (ml = head-within-half)
                    kt_sb[hh] = kvp.tile([P, 4, 8, Q], F8D, tag="ktsb",
                                         name=f"ktsb{hh}")
                    nc.sync.dma_start(
                        kt_sb[hh],
                        ka[hh].rearrange("r (m p) q -> p r m q", p=P),
                    )
                    # v_sb: [p, jj, d-in-half]; jj = 8u+2r+s from row
                    # 256u+128s+p of rank r  (4 DMAs: dims must stay <=3)
                    v_sb[hh] = kvp.tile([P, NB, D // 2], F8D, tag="vsb",
                                        name=f"vsb{hh}")
                    vdst = v_sb[hh].rearrange("p (u r s) d -> p u r s d",
                                              u=2, r=4)
                    for u in range(2):
                        for s2 in range(2):
                            row0 = 256 * u + 128 * s2
                            nc.sync.dma_start(
                                vdst[:, u, :, s2, :],
                                va[hh][:, row0:row0 + P, :].rearrange(
                                    "r p d -> p r d"),
                            )

                ESC = ISQ / (WS * WS)
                for h in range(H):
                    hh, hl = h // 8, h % 8
                    e8 = eap.tile([P, NB, Q], F8D, tag="e8")
                    # pass A, k-tiles 0..7 (q cols 256: always causal-visible
                    # -> exp straight to fp8; cols :256 masked via DVE)
                    for j in range(4):
                        sp = psA.tile([P, 2, Q], FP, tag="spair")
                        for s in range(2):
                            nc.tensor.matmul(
                                sp[:, s, :],
                                kt_sb[hh][:, j % 4, hl,
                                          128 * s:128 * s + P],
                                qt8[:, h, :], start=True, stop=True,
                            )
                        ebf = workp.tile([P, 2, BLK], BF, tag="ebf")
                        nc.scalar.activation(ebf, sp[:, :, :BLK], AF.Exp,
                                             bias=nln16, scale=ESC)
                        nc.scalar.activation(e8[:, 2 * j:2 * j + 2, BLK:],
                                             sp[:, :, BLK:], AF.Exp,
                                             bias=nln16, scale=ESC)
                        nc.vector.tensor_mul(e8[:, 2 * j:2 * j + 2, :BLK],
                                             ebf, mask_sb[:, 2 * j:2 * j + 2, :])
                    # pass A, k-tiles 8..15 (only q cols 256: of block g+4)
                    for j in range(4, 8):
                        sp2 = psB.tile([P, 2, BLK], FP, tag="sp2")
                        for s in range(2):
                            nc.tensor.matmul(
                                sp2[:, s, :],
                                kt_sb[hh][:, j % 4, hl,
                                          BLK + 128 * s:BLK + 128 * s + P],
                                qt8[:, h, BLK:],
                                start=(s == 0), stop=(s == 1),
                                skip_group_check=True,
                            )
                        ebf = workp.tile([P, 2, BLK], BF, tag="ebf")
                        nc.scalar.activation(ebf, sp2, AF.Exp,
                                             bias=nln16, scale=ESC)
                        nc.vector.tensor_mul(e8[:, 2 * j:2 * j + 2, :BLK],
                                             ebf, mask_sb[:, 2 * j:2 * j + 2, :])
                    # pass B: PV + denominator via fp8 DoubleRow
                    aps = psacc.tile([P, Q], FP, tag="acc")
                    dps = psacc.tile([P, Q], FP, tag="acc")
                    for j in range(NB // 2):
                        jj = 2 * j
                        n = Q if jj < 8 else BLK
                        osl = slice(Q - n, Q)
                        b2 = jj // 2
                        r, row0 = b2 % 4, 256 * (b2 // 4)
                        nc.tensor.matmul(
                            aps[:, osl],
                            v_sb[hh][:, jj:jj + 2,
                                     hl * HD:(hl + 1) * HD],
                            e8[:, jj:jj + 2, :n],
                            start=(j == 0), stop=(j == NB // 2 - 1),
                            perf_mode=DR,
                        )
                        nc.tensor.matmul(
                            dps[:, osl], ones8, e8[:, jj:jj + 2, :n],
                            start=(j == 0), stop=(j == NB // 2 - 1),
                            perf_mode=DR,
                        )
                    rec = statp.tile([P, Q], FP, tag="rb")
                    nc.vector.reciprocal(rec, dps)
                    nc.vector.tensor_mul(attn8[:, h, :], aps, rec)

            # ---- phase 6: o-projection + residual (into ht_sb) ----
            with ExitStack() as ctx:
                wp = ctx.enter_context(tc.tile_pool(name="wo", bufs=3))
                pmo = ctx.enter_context(tc.tile_pool(name="pmo", bufs=4,
                                                     space="PSUM"))
                for mt in range(KO):
                    wt = wp.tile([P, KO, P], F8D, tag="wqk")
                    nc.sync.dma_start(wt, wot_d[:, mt, :, :])
                    ps = pmo.tile([P, Q], FP, tag="mm")
                    for j in range(KO // 2):
                        nc.tensor.matmul(
                            ps, wt[:, 2 * j:2 * j + 2, :],
                            attn8[:, 2 * j:2 * j + 2, :],
                            start=(j == 0), stop=(j == KO // 2 - 1),
                            perf_mode=DR,
                        )
                    ot = workp.tile([P, Q], FP, tag="ot")
                    nc.scalar.activation(ot, ps, AF.Copy,
                                         scale=1.0 / (WS * ATS))
                    nc.vector.tensor_add(ht_sb[:, mt, :], ot, ht_sb[:, mt, :])

        # ---- phase 7: norm2 (bf16 out for the MLP, split per k-tile) ----
        with ExitStack() as ctx:
            ynp = ctx.enter_context(tc.tile_pool(name="yn", bufs=1))
            pml = ctx.enter_context(tc.tile_pool(name="pml", bufs=4,
                                                 space="PSUM"))
            ynb = [ynp.tile([P, Q], BF, tag=f"yn{ko}", name=f"yn{ko}")
                   for ko in range(KO)]
            norm(ht_sb, lnw2,
                 lambda ko, tmp: nc.scalar.activation(
                     ynb[ko], tmp, AF.Copy, scale=lnw2[:, ko:ko + 1]))

            # ---- phases 8+9: MLP (bf16) ----
            with ExitStack() as ctx2:
                wgp = ctx2.enter_context(tc.tile_pool(name="wgu", bufs=2))
                wdp = ctx2.enter_context(tc.tile_pool(name="wdp", bufs=2))
                hp = ctx2.enter_context(tc.tile_pool(name="hmid", bufs=1))
                hmid = hp.tile([P, MF, Q], BF, tag="hmid")
                for mf in range(MF):
                    wg_t = wgp.tile([P, KO, P], BF, tag="wg")
                    nc.sync.dma_start(wg_t, wgt_d[:, mf, :, :])
                    wu_t = wgp.tile([P, KO, P], BF, tag="wu")
                    nc.sync.dma_start(wu_t, wut_d[:, mf, :, :])
                    gps = pml.tile([P, Q], FP, tag="mm")
                    ups = pml.tile([P, Q], FP, tag="mm")
                    for ko in range(KO):
                        nc.tensor.matmul(gps, wg_t[:, ko, :], ynb[ko],
                                         start=(ko == 0), stop=(ko == KO - 1))
                    for ko in range(KO):
                        nc.tensor.matmul(ups, wu_t[:, ko, :], ynb[ko],
                                         start=(ko == 0), stop=(ko == KO - 1))
                    sil = workp.tile([P, Q], BF, tag="sil")
                    nc.scalar.activation(sil, gps, AF.Silu)
                    nc.vector.tensor_mul(hmid[:, mf, :], sil, ups)

                for mt in range(KO):
                    wd_t = wdp.tile([P, MF, P], BF, tag="wd")
                    nc.sync.dma_start(wd_t, wdt_d[:, mt, :, :])
                    ps = pml.tile([P, Q], FP, tag="mm")
                    for kf in range(MF):
                        nc.tensor.matmul(ps, wd_t[:, kf, :], hmid[:, kf, :],
                                         start=(kf == 0), stop=(kf == MF - 1))
                    ot = workp.tile([P, Q], FP, tag="ot")
                    nc.vector.tensor_add(ot, ps, ht_sb[:, mt, :])
                    nc.sync.dma_start(out_d[:, mt, :], ot)

    nc.compile()
    return nc


_NC_CACHE = None


def kernel(**inputs) -> np.ndarray:
    global _NC_CACHE
    hidden = np.asarray(inputs["hidden_states"])
    shared = prepare_shared(
        np.asarray(inputs["wq"]), np.asarray(inputs["wk"]),
        np.asarray(inputs["wv"]), np.asarray(inputs["wo"]),
        np.asarray(inputs["w_gate"]), np.asarray(inputs["w_up"]),
        np.asarray(inputs["w_down"]), np.asarray(inputs["ln1_w"]),
        np.asarray(inputs["ln2_w"]),
    )
    in_maps = []
    for core in range(NCORES):
        m = dict(shared)
        m.update(prepare_core(np.asarray(hidden, dtype=np.float32), core))
        in_maps.append(m)

    from concourse.bass_utils import run_bass_kernel_spmd

    if _NC_CACHE is None:
        _NC_CACHE = _build_bass()
    nc = _NC_CACHE
    trace = bool(int(os.environ.get("KERNEL_TRACE", "0")))
    res = run_bass_kernel_spmd(
        nc, in_maps, core_ids=list(range(NCORES)), trace=trace
    )
    if trace and res.exec_time_ns is not None:
        print(f"HW exec time: {res.exec_time_ns} ns")
    outs = [res.results[c]["out"] for c in range(NCORES)]
    return assemble(outs, hidden.dtype)


# revision 24
# speedup vs baseline: 1.2073x; 1.0095x over previous
"""BitNetV3 transformer block on 8 Trainium2 NeuronCores.

Sharding: sequence-parallel. Each core owns 512 query tokens (two
256-token blocks (g, g+4) of one batch element; cores 0-3 -> batch 0,
cores 4-7 -> batch 1). Weights are replicated and host-pre-transposed;
the attention path (q/k/v/o projections, scores, PV) runs in fp8-e4m3
with DoubleRow pairing on the contraction dimension, the MLP stays
bf16. K and V are exchanged with four 4-rank fp8 AllGathers (K half,
V half per 1024-d slice) pipelined behind the projection compute.
Causal masking uses host-supplied per-core 0/1 fp8 mask tiles so the
SPMD program is identical on every core. Activations live transposed
([d, token]); per-token reductions use ones-vector (DoubleRow) matmuls
onto all partitions.

Scale conventions (fp8 ranges): weights wq/wk/wv/wo are stored x32;
q/k/v circulate x32; e = exp(s*ISQ)/16 (fits fp8 max 240); attention
probabilities circulate x8; o-projection PSUM is x256 and is unscaled
by a ScalarE copy before the residual add.
"""

import os
from contextlib import ExitStack

import numpy as np
import ml_dtypes

# ---- problem constants (hardcoded per the harness contract) ----
B = 2
S = 2048
D = 2048
H = 16
HD = 128
DFF = 8192
EPS = 1e-6
ISQ = float(1.0 / np.sqrt(HD))

P = 128  # partitions
KO = D // P  # 16 d-tiles
Q = 512  # tokens per core
NB = S // P  # 16 k-tiles per batch
MF = DFF // P  # 64 dff-tiles
BLK = 256  # token block
NCORES = 8

WS = 32.0  # fp8 weight scale for wq/wk/wv/wo
ESH = float(np.log(16.0))  # e = exp(s) / 16
ATS = 8.0  # attn prob scale in fp8

BF16 = ml_dtypes.bfloat16
F8 = ml_dtypes.float8_e4m3  # TRN FP8_EXP4 (bias 7, max 240)


# ---------------------------------------------------------------------------
# Host-side data preparation (sharding + layout)
# ---------------------------------------------------------------------------

def _w5(w_t: np.ndarray, kt: int, mt: int, dtype, scale=1.0) -> np.ndarray:
    """[K, M] (transposed weight, contraction-major) -> [128, mt, kt, 128]
    with W5[p, m, k, i] = w_t[k*128+p, m*128+i]."""
    K, M = w_t.shape
    assert K == kt * P and M == mt * P
    return np.ascontiguousarray(
        (w_t * scale).reshape(kt, P, mt, P).transpose(1, 2, 0, 3)
    ).astype(dtype)


def _core_tokens(g: int) -> np.ndarray:
    t1 = np.arange(BLK * g, BLK * (g + 1))
    t2 = np.arange(BLK * (g + 4), BLK * (g + 5))
    return np.concatenate([t1, t2])


def _core_mask(g: int) -> np.ndarray:
    """[128, NB, 256] fp8 causal 0/1 mask for the maskable region only.

    jj < 8: cols = q tokens of block g (device cols 0..255; cols 256..511,
    block g+4, are always causally visible for k-blocks 0..3).
    jj >= 8: cols = q tokens of block g+4 (shifted layout)."""
    toks = _core_tokens(g)
    m = np.zeros((P, NB, BLK), dtype=np.float32)
    for jj in range(NB):
        kk = 128 * jj + np.arange(P)
        if jj < 8:
            m[:, jj, :] = kk[:, None] <= toks[None, :BLK]
        else:
            m[:, jj, :] = kk[:, None] <= toks[None, BLK:]
    return m.astype(F8)


def prepare_shared(wq, wk, wv, wo, w_gate, w_up, w_down, ln1_w, ln2_w):
    d = {}
    d["wqt"] = _w5(np.ascontiguousarray(wq.T), KO, KO, F8, WS)
    d["wkt"] = _w5(np.ascontiguousarray(wk.T), KO, KO, F8, WS)
    d["wot"] = _w5(np.ascontiguousarray(wo.T), KO, KO, F8, WS)
    # V projection rhs layout: [128, ko, 2048]
    d["wvt"] = np.ascontiguousarray(
        (wv.T * WS).reshape(KO, P, D).transpose(1, 0, 2)
    ).astype(F8)
    d["wgt"] = _w5(np.ascontiguousarray(w_gate.T), KO, MF, BF16)
    d["wut"] = _w5(np.ascontiguousarray(w_up.T), KO, MF, BF16)
    d["wdt"] = _w5(np.ascontiguousarray(w_down.T), MF, KO, BF16)
    d["ln1"] = np.ascontiguousarray(ln1_w.reshape(KO, P).T).astype(np.float32)
    d["ln2"] = np.ascontiguousarray(ln2_w.reshape(KO, P).T).astype(np.float32)
    return d


def prepare_core(hidden, core: int):
    b, g = core // 4, core % 4
    toks = _core_tokens(g)
    ht = hidden[b][toks].T  # [2048 d, 512 q]
    ht5 = np.ascontiguousarray(ht.reshape(KO, P, Q).transpose(1, 0, 2)).astype(
        np.float32
    )
    return {"ht": ht5, "mask": _core_mask(g)}


def assemble(outs, hidden_dtype):
    full = np.empty((B, S, D), dtype=np.float32)
    for core in range(NCORES):
        b, g = core // 4, core % 4
        toks = _core_tokens(g)
        o = np.asarray(outs[core])  # [p, ko, q]
        full[b, toks, :] = o.transpose(2, 1, 0).reshape(Q, D)
    return full.astype(hidden_dtype)


def _kslice(jj: int):
    """Global k-tile jj -> (rank r, col offset) in kt_all[r, ml, p, q]-style
    gathered K buffer (rank-major: rank r holds token blocks r and r+4)."""
    b, s = jj // 2, jj % 2
    return b % 4, 256 * (b // 4) + 128 * s


# ---------------------------------------------------------------------------
# Pure-numpy simulation of the exact device dataflow (for fast validation)
# ---------------------------------------------------------------------------

def _bf(x):
    return x.astype(BF16).astype(np.float32)


def _f8(x):
    return np.clip(np.asarray(x, np.float32), -240.0, 240.0).astype(F8).astype(
        np.float32
    )


def _sim_norm(ht, lnw, f8out):
    sq = _f8(ht * ht)
    ms = sq.sum(axis=0)
    rstd = 1.0 / np.sqrt(ms / D + EPS)
    o = (ht * rstd[None, :]) * lnw[:, None]
    return _f8(o) if f8out else _bf(o)


def host_simulate(inputs):
    hidden = np.asarray(inputs["hidden_states"], dtype=np.float32)
    f32 = lambda k: np.asarray(inputs[k], dtype=np.float32)  # noqa: E731
    wqT = _f8(f32("wq").T * WS)
    wkT = _f8(f32("wk").T * WS)
    wvT = _f8(f32("wv").T * WS)
    woT = _f8(f32("wo").T * WS)
    wgT, wuT, wdT = _bf(f32("w_gate").T), _bf(f32("w_up").T), _bf(f32("w_down").T)
    ln1, ln2 = f32("ln1_w"), f32("ln2_w")

    kts, vs, xns, hts = {}, {}, {}, {}
    for core in range(NCORES):
        b, g = core // 4, core % 4
        ht = hidden[b][_core_tokens(g)].T  # [2048, 512]
        hts[core] = ht
        xn = _sim_norm(ht, ln1, f8out=True)
        xns[core] = xn
        kts[core] = _f8(wkT.T @ xn)  # 32*kT [2048, 512]
        vs[core] = _f8(xn.T @ wvT)  # 32*v natural [512, 2048]

    outs = []
    for core in range(NCORES):
        b, g = core // 4, core % 4
        grp = [4 * b + r for r in range(4)]
        kt_all = np.stack([kts[c] for c in grp])  # [4, 2048, 512]
        v_all = np.stack([vs[c] for c in grp])  # [4, 512, 2048]
        mask = np.asarray(_core_mask(g), dtype=np.float32)

        xn = xns[core]
        qT = _f8(wqT.T @ xn)  # 32*qT [2048, 512]
        attn8 = np.zeros((D, Q), dtype=np.float32)
        for h in range(H):
            aps = np.zeros((HD, Q), dtype=np.float32)
            den = np.zeros(Q, dtype=np.float32)
            for jj in range(NB):
                r, off = _kslice(jj)
                n = Q if jj < 8 else BLK
                kth = kt_all[r, h * HD:(h + 1) * HD, off:off + P]  # [hd,128]
                sc = kth.T @ qT[h * HD:(h + 1) * HD, Q - n:]
                ex = np.exp(sc * (ISQ / (WS * WS)) - ESH)
                e = np.concatenate(
                    [_f8(ex[:, :BLK] * mask[:, jj, :]), _f8(ex[:, BLK:])],
                    axis=1)
                vt = v_all[r, off:off + P, h * HD:(h + 1) * HD]  # [128, hd]
                aps[:, Q - n:] += vt.T @ e
                den[Q - n:] += e.sum(axis=0)
            rec = 1.0 / (4.0 * den)
            attn8[h * HD:(h + 1) * HD] = _f8(aps * rec[None, :])
        ops = woT.T @ attn8  # 256*o
        h2 = hts[core] + ops / (WS * ATS)
        yT = _sim_norm(h2, ln2, f8out=False)
        gate = wgT.T @ yT
        up = wuT.T @ yT
        sil = _bf(gate / (1.0 + np.exp(-gate)))
        hmid = _bf(sil * up)
        outT = h2 + wdT.T @ hmid
        outs.append(outT.reshape(KO, P, Q).transpose(1, 0, 2).astype(np.float32))
    return assemble(outs, np.asarray(inputs["hidden_states"]).dtype)


# ---------------------------------------------------------------------------
# Device program
# ---------------------------------------------------------------------------

def _build_bass():
    import concourse.bacc as bacc
    import concourse.mybir as mybir
    import concourse.tile as tile

    FP = mybir.dt.float32
    BF = mybir.dt.bfloat16
    F8D = mybir.dt.float8e4
    AF = mybir.ActivationFunctionType
    DR = mybir.MatmulPerfMode.DoubleRow

    nc = bacc.Bacc("TRN2", target_bir_lowering=False, debug=False,
                   num_devices=NCORES)

    ht_d = nc.dram_tensor("ht", [P, KO, Q], FP, kind="ExternalInput")
    mask_d = nc.dram_tensor("mask", [P, NB, BLK], F8D, kind="ExternalInput")
    ln1_d = nc.dram_tensor("ln1", [P, KO], FP, kind="ExternalInput")
    ln2_d = nc.dram_tensor("ln2", [P, KO], FP, kind="ExternalInput")
    wqt_d = nc.dram_tensor("wqt", [P, KO, KO, P], F8D, kind="ExternalInput")
    wkt_d = nc.dram_tensor("wkt", [P, KO, KO, P], F8D, kind="ExternalInput")
    wvt_d = nc.dram_tensor("wvt", [P, KO, D], F8D, kind="ExternalInput")
    wot_d = nc.dram_tensor("wot", [P, KO, KO, P], F8D, kind="ExternalInput")
    wgt_d = nc.dram_tensor("wgt", [P, MF, KO, P], BF, kind="ExternalInput")
    wut_d = nc.dram_tensor("wut", [P, MF, KO, P], BF, kind="ExternalInput")
    wdt_d = nc.dram_tensor("wdt", [P, KO, MF, P], BF, kind="ExternalInput")
    out_d = nc.dram_tensor("out", [P, KO, Q], FP, kind="ExternalOutput")

    groups = [[0, 1, 2, 3], [4, 5, 6, 7]]

    with tile.TileContext(nc) as tc, ExitStack() as top:
        dramp = top.enter_context(tc.tile_pool(name="dram", bufs=1, space="DRAM"))
        constp = top.enter_context(tc.tile_pool(name="const", bufs=1))
        statp = top.enter_context(tc.tile_pool(name="stat", bufs=2))
        workp = top.enter_context(tc.tile_pool(name="work", bufs=2))
        # 2 PSUM banks for long-lived accumulators (norm ms / attn aps+dps)
        psacc = top.enter_context(tc.tile_pool(name="psacc", bufs=2,
                                               space="PSUM"))

        # value 4.0: folds the x4 denominator unscale into the ones-matmul
        # (attn8 = aps / (4*den')); norm() compensates via its Sqrt scale.
        ones8 = constp.tile([P, 2, P], F8D, tag="ones8")
        nc.vector.memset(ones8, 4.0)
        eps_t = constp.tile([P, 1], FP, tag="eps")
        nc.vector.memset(eps_t, EPS)
        nln16 = constp.tile([P, 1], FP, tag="nln16")
        nc.vector.memset(nln16, -ESH)
        lnw1 = constp.tile([P, KO], FP, tag="ln1")
        nc.sync.dma_start(lnw1, ln1_d[:])
        lnw2 = constp.tile([P, KO], FP, tag="ln2")
        nc.sync.dma_start(lnw2, ln2_d[:])

        ht_sb = constp.tile([P, KO, Q], FP, tag="ht")  # becomes h2 in place
        for c in range(4):
            nc.sync.dma_start(ht_sb[:, 4 * c:4 * c + 4, :],
                              ht_d[:, 4 * c:4 * c + 4, :])
        # x_norm^T fp8, split per contraction pair so consumers can start
        # before the full norm completes
        xn8p = [constp.tile([P, 2, Q], F8D, tag=f"xn8_{j}", name=f"xn8_{j}")
                for j in range(KO // 2)]

        # k bounce/gathered: [ml*128+p, q] per half; v: [tok, 1024 d] per half
        kb = [dramp.tile([8 * P, Q], F8D, name=f"kb{i}") for i in range(2)]
        ka = [dramp.tile([4, 8 * P, Q], F8D, name=f"ka{i}") for i in range(2)]
        vb = [dramp.tile([Q, D // 2], F8D, name=f"vb{i}") for i in range(2)]
        va = [dramp.tile([4, Q, D // 2], F8D, name=f"va{i}") for i in range(2)]

        def norm(src, lnw, dst_fn):
            # mean-square via squares (alternating ScalarE/DVE to halve the
            # serial latency) + fp8 DoubleRow ones-matmuls; normalize =
            # DVE mul (x*rstd) + ScalarE per-partition ln scale
            ms = psacc.tile([P, Q], FP, tag="acc")
            for j in range(KO // 2):
                sq = workp.tile([P, 2, Q], F8D, tag="sq")
                src_j = src[:, 2 * j:2 * j + 2, :]
                if j % 2 == 0:
                    nc.scalar.activation(sq, src_j, AF.Square)
                else:
                    nc.vector.tensor_mul(sq, src_j, src_j)
                nc.tensor.matmul(ms, ones8, sq, start=(j == 0),
                                 stop=(j == KO // 2 - 1), perf_mode=DR)
            st = statp.tile([P, Q], FP, tag="st")
            nc.scalar.activation(st, ms, AF.Sqrt, bias=eps_t,
                                 scale=1.0 / (4.0 * D))
            rb = statp.tile([P, Q], FP, tag="rb")
            nc.vector.reciprocal(rb, st)
            for ko in range(KO):
                tmp = workp.tile([P, Q], FP, tag="nrm")
                nc.vector.tensor_mul(tmp, src[:, ko, :], rb)
                dst_fn(ko, tmp)

        # ---- phase 1: norm1 ----
        norm(ht_sb, lnw1,
             lambda ko, tmp: nc.scalar.activation(
                 xn8p[ko // 2][:, ko % 2, :], tmp, AF.Copy,
                 scale=lnw1[:, ko:ko + 1]))

        with ExitStack() as mid:
            midp = mid.enter_context(tc.tile_pool(name="mid", bufs=1))
            qt8 = midp.tile([P, KO, Q], F8D, tag="qt8")
            attn8 = midp.tile([P, KO, Q], F8D, tag="attn8")
            maskp = mid.enter_context(tc.tile_pool(name="maskp", bufs=1))
            mask_sb = maskp.tile([P, NB, BLK], F8D, tag="mask")
            nc.sync.dma_start(mask_sb, mask_d[:])
            # gathered-K/V SBUF tiles: pool created BEFORE the weight pools
            # so its addresses don't overlap them (else the kt_sb DMA picks
            # up a write-after-read dep on the projection matmuls)
            kvp = mid.enter_context(tc.tile_pool(name="kvp", bufs=2))
            kt_sb = {}
            v_sb = {}
            with ExitStack() as ctx:
                wkp = ctx.enter_context(tc.tile_pool(name="wkp", bufs=8))
                wqp = ctx.enter_context(tc.tile_pool(name="wqp", bufs=12))
                wvp = ctx.enter_context(tc.tile_pool(name="wvp", bufs=2))
                pmm = ctx.enter_context(tc.tile_pool(name="pmm", bufs=4,
                                                     space="PSUM"))
                # prefetch qkv weights in first-use order (DMA queues are
                # idle now; later they compete with the AllGathers)
                wk_t, wq_t, wv_t = {}, {}, {}

                def wfetch(d, mt, pool, w, pfx, shape=None):
                    t = pool.tile(shape or [P, KO, P], F8D, tag=pfx,
                                  name=f"{pfx}{mt}")
                    nc.sync.dma_start(t, d)
                    w[mt] = t

                for mt in range(8):
                    wfetch(wkt_d[:, mt, :, :], mt, wkp, wk_t, "wk")
                for mt in range(8):
                    wfetch(wqt_d[:, mt, :, :], mt, wqp, wq_t, "wq")
                for n in range(2):
                    wfetch(wvt_d[:, :, n * Q:(n + 1) * Q], n, wvp, wv_t,
                           "wv", [P, KO, Q])
                for mt in range(8, KO):
                    wfetch(wqt_d[:, mt, :, :], mt, wqp, wq_t, "wq")
                for mt in range(8, KO):
                    wfetch(wkt_d[:, mt, :, :], mt, wkp, wk_t, "wk")
                for n in range(2, 4):
                    wfetch(wvt_d[:, :, n * Q:(n + 1) * Q], n, wvp, wv_t,
                           "wv", [P, KO, Q])

                def proj_t(w_tiles, dst_fn, lo=0, hi=KO):
                    for mt in range(lo, hi):
                        wt = w_tiles[mt]
                        ps = pmm.tile([P, Q], FP, tag="mm")
                        for j in range(KO // 2):
                            nc.tensor.matmul(
                                ps, wt[:, 2 * j:2 * j + 2, :], xn8p[j],
                                start=(j == 0), stop=(j == KO // 2 - 1),
                                perf_mode=DR,
                            )
                        dst_fn(mt, ps)

                def k_out(mt, ps):
                    stg = workp.tile([P, Q], F8D, tag="stg")
                    nc.vector.tensor_copy(stg, ps)
                    ml = mt % 8
                    nc.sync.dma_start(kb[mt // 8][ml * P:(ml + 1) * P, :], stg)

                def v_chunk(n):
                    wv_c = wv_t[n]
                    for tt in range(4):
                        ps = pmm.tile([P, Q], FP, tag="mm")
                        for j in range(KO // 2):
                            nc.tensor.matmul(
                                ps,
                                xn8p[j][:, :, tt * P:(tt + 1) * P],
                                wv_c[:, 2 * j:2 * j + 2, :],
                                start=(j == 0), stop=(j == KO // 2 - 1),
                                perf_mode=DR,
                            )
                        stg = workp.tile([P, Q], F8D, tag="stg")
                        nc.vector.tensor_copy(stg, ps)
                        nc.sync.dma_start(
                            vb[n // 2][tt * P:(tt + 1) * P,
                                       (n % 2) * Q:(n % 2 + 1) * Q],
                            stg,
                        )

                def ag(inb, outb, pat):
                    nc.gpsimd.collective_compute(
                        "AllGather", mybir.AluOpType.bypass,
                        ins=[inb.opt()],
                        outs=[outb.rearrange(pat).opt()],
                        replica_groups=groups,
                    )

                def load_kt(hh):
                    # kt_sb: [p, r, ml, q]   (ml = head-within-half)
                    kt_sb[hh] = kvp.tile([P, 4, 8, Q], F8D, tag="ktsb",
                                         name=f"ktsb{hh}")
                    nc.sync.dma_start(
                        kt_sb[hh],
                        ka[hh].rearrange("r (m p) q -> p r m q", p=P),
                    )

                def load_v(hh):
                    # v_sb: [p, jj, d-in-half]; jj = 8u+2r+s from row
                    # 256u+128s+p of rank r  (4 DMAs: dims must stay <=3)
                    v_sb[hh] = kvp.tile([P, NB, D // 2], F8D, tag="vsb",
                                        name=f"vsb{hh}")
                    vdst = v_sb[hh].rearrange("p (u r s) d -> p u r s d",
                                              u=2, r=4)
                    for u in range(2):
                        for s2 in range(2):
                            row0 = 256 * u + 128 * s2
                            nc.sync.dma_start(
                                vdst[:, u, :, s2, :],
                                va[hh][:, row0:row0 + P, :].rearrange(
                                    "r p d -> p r d"),
                            )

                # pipeline: K0 -> AGk0 | Q | V0 -> AGv0 | K1 -> AGk1 |
                #           V1 -> AGv1  (attention h0-7 needs k0+v0 first)
                proj_t(wk_t, k_out, 0, 8)
                ag(kb[0], ka[0], "r m q -> (r m) q")
                load_kt(0)
                proj_t(wq_t, lambda mt, ps:
                       nc.vector.tensor_copy(qt8[:, mt, :], ps))
                v_chunk(0)
                v_chunk(1)
                ag(vb[0], va[0], "r t d -> (r t) d")
                load_v(0)
                proj_t(wk_t, k_out, 8, 16)
                ag(kb[1], ka[1], "r m q -> (r m) q")
                load_kt(1)
                v_chunk(2)
                v_chunk(3)
                ag(vb[1], va[1], "r t d -> (r t) d")
                load_v(1)

            # ---- phase 5: attention ----
            with ExitStack() as ctx:
                eap = ctx.enter_context(tc.tile_pool(name="eap", bufs=2))
                psA = ctx.enter_context(tc.tile_pool(name="psA", bufs=2,
                                                     space="PSUM"))
                psB = ctx.enter_context(tc.tile_pool(name="psB", bufs=2,
                                                     space="PSUM"))
                ESC = ISQ / (WS * WS)
                for h in range(H):
                    hh, hl = h // 8, h % 8
                    e8 = eap.tile([P, NB, Q], F8D, tag="e8")
                    # pass A, k-tiles 0..7 (q cols 256: always causal-visible
                    # -> exp straight to fp8; cols :256 masked via DVE)
                    for j in range(4):
                        sp = psA.tile([P, 2, Q], FP, tag="spair")
                        for s in range(2):
                            nc.tensor.matmul(
                                sp[:, s, :],
                                kt_sb[hh][:, j % 4, hl,
                                          128 * s:128 * s + P],
                                qt8[:, h, :], start=True, stop=True,
                            )
                        ebf = workp.tile([P, 2, BLK], BF, tag="ebf")
                        nc.scalar.activation(ebf, sp[:, :, :BLK], AF.Exp,
                                             bias=nln16, scale=ESC)
                        nc.scalar.activation(e8[:, 2 * j:2 * j + 2, BLK:],
                                             sp[:, :, BLK:], AF.Exp,
                                             bias=nln16, scale=ESC)
                        nc.vector.tensor_mul(e8[:, 2 * j:2 * j + 2, :BLK],
                                             ebf, mask_sb[:, 2 * j:2 * j + 2, :])
                    # pass A, k-tiles 8..15 (only q cols 256: of block g+4)
                    for j in range(4, 8):
                        sp2 = psB.tile([P, 2, BLK], FP, tag="sp2")
                        for s in range(2):
                            nc.tensor.matmul(
                                sp2[:, s, :],
                                kt_sb[hh][:, j % 4, hl,
                                          BLK + 128 * s:BLK + 128 * s + P],
                                qt8[:, h, BLK:],
                                start=(s == 0), stop=(s == 1),
                                skip_group_check=True,
                            )
                        ebf = workp.tile([P, 2, BLK], BF, tag="ebf")
                        nc.scalar.activation(ebf, sp2, AF.Exp,
                                             bias=nln16, scale=ESC)
                        nc.vector.tensor_mul(e8[:, 2 * j:2 * j + 2, :BLK],
                                             ebf, mask_sb[:, 2 * j:2 * j + 2, :])
                    # pass B: PV + denominator via fp8 DoubleRow
                    aps = psacc.tile([P, Q], FP, tag="acc")
                    dps = psacc.tile([P, Q], FP, tag="acc")
                    for j in range(NB // 2):
                        jj = 2 * j
                        n = Q if jj < 8 else BLK
                        osl = slice(Q - n, Q)
                        b2 = jj // 2
                        r, row0 = b2 % 4, 256 * (b2 // 4)
                        nc.tensor.matmul(
                            aps[:, osl],
                            v_sb[hh][:, jj:jj + 2,
                                     hl * HD:(hl + 1) * HD],
                            e8[:, jj:jj + 2, :n],
                            start=(j == 0), stop=(j == NB // 2 - 1),
                            perf_mode=DR,
                        )
                        nc.tensor.matmul(
                            dps[:, osl], ones8, e8[:, jj:jj + 2, :n],
                            start=(j == 0), stop=(j == NB // 2 - 1),
                            perf_mode=DR,
                        )
                    rec = statp.tile([P, Q], FP, tag="rb")
                    nc.vector.reciprocal_approx_fast(rec, dps)
                    nc.vector.tensor_mul(attn8[:, h, :], aps, rec)

            # ---- phase 6: o-projection + residual (into ht_sb) ----
            with ExitStack() as ctx:
                wp = ctx.enter_context(tc.tile_pool(name="wo", bufs=3))
                pmo = ctx.enter_context(tc.tile_pool(name="pmo", bufs=4,
                                                     space="PSUM"))
                for mt in range(KO):
                    wt = wp.tile([P, KO, P], F8D, tag="wqk")
                    nc.sync.dma_start(wt, wot_d[:, mt, :, :])
                    ps = pmo.tile([P, Q], FP, tag="mm")
                    for j in range(KO // 2):
                        nc.tensor.matmul(
                            ps, wt[:, 2 * j:2 * j + 2, :],
                            attn8[:, 2 * j:2 * j + 2, :],
                            start=(j == 0), stop=(j == KO // 2 - 1),
                            perf_mode=DR,
                        )
                    ot = workp.tile([P, Q], FP, tag="ot")
                    nc.scalar.activation(ot, ps, AF.Copy,
                                         scale=1.0 / (WS * ATS))
                    nc.vector.tensor_add(ht_sb[:, mt, :], ot, ht_sb[:, mt, :])

        # ---- phase 7: norm2 (bf16 out for the MLP, split per k-tile) ----
        with ExitStack() as ctx:
            ynp = ctx.enter_context(tc.tile_pool(name="yn", bufs=1))
            pml = ctx.enter_context(tc.tile_pool(name="pml", bufs=4,
                                                 space="PSUM"))
            ynb = [ynp.tile([P, Q], BF, tag=f"yn{ko}", name=f"yn{ko}")
                   for ko in range(KO)]
            norm(ht_sb, lnw2,
                 lambda ko, tmp: nc.scalar.activation(
                     ynb[ko], tmp, AF.Copy, scale=lnw2[:, ko:ko + 1]))

            # ---- phases 8+9: MLP (bf16) ----
            with ExitStack() as ctx2:
                wgp = ctx2.enter_context(tc.tile_pool(name="wgu", bufs=2))
                wdp = ctx2.enter_context(tc.tile_pool(name="wdp", bufs=2))
                hp = ctx2.enter_context(tc.tile_pool(name="hmid", bufs=1))
                hmid = hp.tile([P, MF, Q], BF, tag="hmid")
                for mf in range(MF):
                    wg_t = wgp.tile([P, KO, P], BF, tag="wg")
                    nc.sync.dma_start(wg_t, wgt_d[:, mf, :, :])
                    wu_t = wgp.tile([P, KO, P], BF, tag="wu")
                    nc.sync.dma_start(wu_t, wut_d[:, mf, :, :])
                    gps = pml.tile([P, Q], FP, tag="mm")
                    ups = pml.tile([P, Q], FP, tag="mm")
                    for ko in range(KO):
                        nc.tensor.matmul(gps, wg_t[:, ko, :], ynb[ko],
                                         start=(ko == 0), stop=(ko == KO - 1))
                    for ko in range(KO):
                        nc.tensor.matmul(ups, wu_t[:, ko, :], ynb[ko],
                                         start=(ko == 0), stop=(ko == KO - 1))
                    sil = workp.tile([P, Q], BF, tag="sil")
                    nc.scalar.activation(sil, gps, AF.Silu)
                    nc.vector.tensor_mul(hmid[:, mf, :], sil, ups)

                for mt in range(KO):
                    wd_t = wdp.tile([P, MF, P], BF, tag="wd")
                    nc.sync.dma_start(wd_t, wdt_d[:, mt, :, :])
                    ps = pml.tile([P, Q], FP, tag="mm")
                    for kf in range(MF):
                        nc.tensor.matmul(ps, wd_t[:, kf, :], hmid[:, kf, :],
                                         start=(kf == 0), stop=(kf == MF - 1))
                    ot = workp.tile([P, Q], FP, tag="ot")
                    nc.vector.tensor_add(ot, ps, ht_sb[:, mt, :])
                    nc.sync.dma_start(out_d[:, mt, :], ot)

    nc.compile()
    return nc


_NC_CACHE = None


def kernel(**inputs) -> np.ndarray:
    global _NC_CACHE
    hidden = np.asarray(inputs["hidden_states"])
    shared = prepare_shared(
        np.asarray(inputs["wq"]), np.asarray(inputs["wk"]),
        np.asarray(inputs["wv"]), np.asarray(inputs["wo"]),
        np.asarray(inputs["w_gate"]), np.asarray(inputs["w_up"]),
        np.asarray(inputs["w_down"]), np.asarray(inputs["ln1_w"]),
        np.asarray(inputs["ln2_w"]),
    )
    in_maps = []
    for core in range(NCORES):
        m = dict(shared)
        m.update(prepare_core(np.asarray(hidden, dtype=np.float32), core))
        in_maps.append(m)

    from concourse.bass_utils import run_bass_kernel_spmd

    if _NC_CACHE is None:
        _NC_CACHE = _build_bass()
    nc = _NC_CACHE
    trace = bool(int(os.environ.get("KERNEL_TRACE", "0")))
    res = run_bass_kernel_spmd(
        nc, in_maps, core_ids=list(range(NCORES)), trace=trace
    )
    if trace and res.exec_time_ns is not None:
        print(f"HW exec time: {res.exec_time_ns} ns")
    outs = [res.results[c]["out"] for c in range(NCORES)]
    return assemble(outs, hidden.dtype)
